# revision 19
# baseline (speedup 1.0000x reference)
"""Causal multi-head attention on 8 Trainium2 NeuronCores.

Problem: nn_Attention_46643344835180
  x: [8, 1024, 768], 12 heads x 64 dh, causal softmax attention + output proj.

Sharding: data-parallel over batch (8 batch elements -> 8 cores, no collectives).

Per-core dataflow (batch element b):
  xT = x_b.T                       PE transposes, 6-wide batches      [768, 1024]
  QT = Wq_cat.T @ xT  (+bq)        heads stacked on partitions        [768, 1024]
  KT = Wk_cat.T @ xT  (+bk)                                           [768, 1024]
  V  = x_b @ Wv_cat   (+bv)        + interleaved ones column          [1024, 12*65]
  per head h, query-chunk qc (512):
    S^T[k,q] = KT_h.T @ QT_h          keys on partitions
    P^T = exp(S^T / 8)                ScalarE, batched over 2 key-blocks
    causal: one wide-mask multiply on the partial columns
    z^T[65,512] += [V_h | 1].T @ P^T  row 64 accumulates the denominator
    ZT_h = z^T[0:64] * approx(1/z^T[64])   (reciprocal straight from psum ->
           gpsimd partition_broadcast -> multiply)
  out = ZT.T @ Wo_cat (+bo)                                           [1024, 768]

Scheduling: PE executes in issue order, so projection matmul chains are kept
in a pending queue and issued one chain at a time between attention score
groups (fills the PE bubbles while ScalarE runs exp).  All projection
PSUM->SBUF drains run on GpSimd so ScalarE does exp only.

Dtype config CFG = (bf_qk, bf_vproj, bf_pv, bf_o) picks bf16 vs f32r per stage.
"""

import sys

sys.path.insert(0, "/opt/trn_rl_repo")

import ml_dtypes
import numpy as np

import concourse.bass as bass
import concourse.mybir as mybir
import concourse.tile as tile
from concourse import bacc
from concourse.bass_utils import run_bass_kernel_spmd

F32 = mybir.dt.float32
F32R = mybir.dt.float32r
BF16 = mybir.dt.bfloat16
AF = mybir.ActivationFunctionType

SEQ = 1024
DM = 768
NH = 12
DH = 64
EPH = DH + 1  # 65: head value dim + denominator ones column
BATCH = 8
NQT = SEQ // 128  # 8 seq tiles of 128
NDT = DM // 128  # 6 d_model tiles
QC = 512  # query chunk (moving dim)
NQC = SEQ // QC  # 2
NVC = 2
VC = DM // NVC  # 384
NHP = NH // 2  # 6 head pairs

# (bf_qk, bf_vproj, bf_pv, bf_o)
CFG = (True, True, True, True)


def _npdt(dt):
    return ml_dtypes.bfloat16 if dt == BF16 else np.float32


def build(with_bq, with_bk, with_bv, with_bo, cfg=CFG):
    bf_qk, bf_vproj, bf_pv, bf_o = cfg
    DT_QK = BF16 if bf_qk else F32R  # wq/wk, QT/KT, scores matmul
    DT_VP = BF16 if bf_vproj else F32R  # wv + V-projection compute
    DT_PV = BF16 if bf_pv else F32R  # V storage, P^T, PV matmul
    DT_O = BF16 if bf_o else F32R  # ZT, wo, output matmul
    DT_MASK = BF16 if bf_pv else F32
    # xT feeds both the QK and V projections as a matmul operand, so it must
    # match those weights' dtype
    assert bf_qk == bf_vproj
    DT_X = BF16 if bf_qk else F32R

    nc = bacc.Bacc("TRN2", target_bir_lowering=False, debug=False)

    x = nc.dram_tensor("x", [SEQ, DM], F32R, kind="ExternalInput")
    wq = nc.dram_tensor("wq", [DM, DM], DT_QK, kind="ExternalInput")
    wk = nc.dram_tensor("wk", [DM, DM], DT_QK, kind="ExternalInput")
    wv = nc.dram_tensor("wv", [DM, DM], DT_VP, kind="ExternalInput")
    wo = nc.dram_tensor("wo", [DM, DM], DT_O, kind="ExternalInput")
    wmask = nc.dram_tensor("wmask", [128, 128], DT_MASK, kind="ExternalInput")
    identin = nc.dram_tensor("identin", [128, 128], F32R, kind="ExternalInput")
    bq = bk = bv = bo = None
    if with_bq:
        bq = nc.dram_tensor("bq", [128, NDT], F32, kind="ExternalInput")
    if with_bk:
        bk = nc.dram_tensor("bk", [128, NDT], F32, kind="ExternalInput")
    if with_bv:
        bv = nc.dram_tensor("bv", [1, DM], F32, kind="ExternalInput")
    if with_bo:
        bo = nc.dram_tensor("bo", [1, DM], F32, kind="ExternalInput")
    out = nc.dram_tensor("out", [SEQ, DM], F32, kind="ExternalOutput")

    with tile.TileContext(nc) as tc:
        with (
            tc.tile_pool(name="persist", bufs=1) as persist,
            tc.tile_pool(name="xn", bufs=6) as xn_pool,
            tc.tile_pool(name="wstream", bufs=1) as w_pool,
            tc.tile_pool(name="wqk", bufs=6) as wqk_pool,
            tc.tile_pool(name="pt", bufs=4) as pt_pool,
            tc.tile_pool(name="small", bufs=2) as small,
            tc.tile_pool(name="outst", bufs=2) as out_pool,
            tc.tile_pool(name="ps_st", bufs=2, space="PSUM") as ps_st,
            tc.tile_pool(name="ps_z", bufs=2, space="PSUM") as ps_z,
            tc.tile_pool(name="ps_mm", bufs=2, space="PSUM") as ps_mm,
        ):
            # ---- identity first (warmup gate), then x (longest startup chain).
            # x s-tiles 0-3 on the sync queue, 4-7 on the gpsimd SWDGE queue so
            # the serial per-queue dma_start issue cost (~1us each) halves.
            ident = persist.tile([128, 128], F32R, tag="ident", name="ident")
            nc.sync.dma_start(out=ident, in_=identin[:, :])
            xn = []
            for s in range(NQT):
                t = xn_pool.tile([128, DM], F32R, tag="xn", name="xn")
                eng = nc.sync if s < 4 else nc.gpsimd
                eng.dma_start(out=t, in_=x[s * 128 : (s + 1) * 128, :])
                xn.append(t)

            # HAM warmup: dummy matmuls while the x DMAs land, so the
            # transposes/projections start at 2.4GHz instead of the cold 1.2GHz
            warm_ps = ps_mm.tile(
                [128, 128], F32, tag="proj", name="warm", padded_shape=[128, QC]
            )
            for _ in range(8):
                nc.tensor.matmul(warm_ps, lhsT=ident, rhs=ident, start=True, stop=True)
            def qk_load(hp, eng=None):
                pair = []
                for wsrc in (wq, wk):
                    t = wqk_pool.tile([128, NDT, 128], DT_QK, tag="wqk", name="wqk")
                    (eng or nc.sync).dma_start(
                        out=t,
                        in_=wsrc.rearrange("(d p) c -> p d c", p=128)[
                            :, :, hp * 128 : (hp + 1) * 128
                        ],
                    )
                    pair.append(t)
                return pair

            # first two head-pairs' weights + wv on the scalar queue, ahead of
            # the mask/bias loads (needed within ~10us; sync is busy with x)
            qk_tiles = {0: qk_load(0, nc.scalar), 1: qk_load(1, nc.scalar)}
            wv_t = w_pool.tile([128, NDT, DM], DT_VP, tag="wv", name="wv")
            nc.scalar.dma_start(
                out=wv_t, in_=wv.rearrange("(d p) c -> p d c", p=128)
            )
            wm_t = persist.tile([128, 128], DT_MASK, tag="wmask", name="wmask")
            nc.scalar.dma_start(out=wm_t, in_=wmask[:, :])

            bias_tiles = {}
            if with_bq:
                t = persist.tile([128, NDT], F32, tag="bq", name="bq")
                nc.scalar.dma_start(out=t, in_=bq[:, :])
                bias_tiles["bq"] = t
            if with_bk:
                t = persist.tile([128, NDT], F32, tag="bk", name="bk")
                nc.scalar.dma_start(out=t, in_=bk[:, :])
                bias_tiles["bk"] = t
            if with_bv:
                t = persist.tile([128, DM], F32, tag="bv", name="bv")
                nc.scalar.dma_start(out=t, in_=bv[0:1, :].to_broadcast((128, DM)))
                bias_tiles["bv"] = t
            if with_bo:
                t = persist.tile([128, DM], F32, tag="bo", name="bo")
                nc.scalar.dma_start(out=t, in_=bo[0:1, :].to_broadcast((128, DM)))
                bias_tiles["bo"] = t

            # ---- persistent activations ----
            xT = persist.tile([128, NDT * SEQ], DT_X, tag="xT", name="xT")
            xTv = xT.rearrange("p (d s) -> p d s", d=NDT)
            QT = [
                persist.tile([128, SEQ], DT_QK, tag=f"QT{d}", name=f"QT{d}")
                for d in range(NHP)
            ]
            KT = [
                persist.tile([128, SEQ], DT_QK, tag=f"KT{d}", name=f"KT{d}")
                for d in range(NHP)
            ]
            V = [
                persist.tile([128, NH * EPH], DT_PV, tag=f"V{s}", name=f"V{s}")
                for s in range(NQT)
            ]
            ones_dt = F32 if DT_PV == F32R else DT_PV
            for s in range(NQT):
                # contiguous memset to 1.0; v_proj then overwrites the value
                # columns, leaving 1s only in each head's denominator column
                nc.vector.memset(V[s].bitcast(ones_dt), 1.0)
            ZT = [
                persist.tile([128, SEQ], DT_O, tag=f"ZT{d}", name=f"ZT{d}")
                for d in range(NDT)
            ]

            # ---- transposes: 6 per psum slot, one DVE copy per s-tile ----
            def transpose_s(s):
                pst = ps_st.tile(
                    [128, NDT * 128], F32R, tag="st", name="tp",
                    padded_shape=[128, 2 * QC],
                )
                for d in range(NDT):
                    nc.tensor.transpose(
                        pst[:, d * 128 : (d + 1) * 128],
                        xn[s][:, d * 128 : (d + 1) * 128],
                        ident,
                    )
                nc.vector.tensor_copy(
                    xTv[:, :, s * 128 : (s + 1) * 128],
                    pst.rearrange("p (d q) -> p d q", d=NDT),
                )

            # ---- projection chains (each returns issue-thunks) ----
            def qk_proj_chains(hp, c):
                def mk(widx, dst, bkey):
                    def thunk():
                        acc = ps_mm.tile([128, QC], F32, tag="proj", name="proj")
                        for d in range(NDT):
                            nc.tensor.matmul(
                                acc,
                                lhsT=qk_tiles[hp][widx][:, d, :],
                                rhs=xTv[:, d, c * QC : (c + 1) * QC],
                                start=(d == 0),
                                stop=(d == NDT - 1),
                            )
                        o = dst[hp][:, c * QC : (c + 1) * QC]
                        if bkey in bias_tiles:
                            nc.vector.tensor_scalar_add(
                                o, acc, bias_tiles[bkey][:, hp : hp + 1]
                            )
                        else:
                            nc.vector.tensor_copy(o, acc)

                    return thunk

                return [mk(0, QT, "bq"), mk(1, KT, "bk")]

            def v_proj_chains(s):
                def mk(cc):
                    def thunk():
                        acc = ps_mm.tile(
                            [128, VC], F32, tag="proj", name="vacc",
                            padded_shape=[128, QC],
                        )
                        for d in range(NDT):
                            nc.tensor.matmul(
                                acc,
                                lhsT=xTv[:, d, s * 128 : (s + 1) * 128],
                                rhs=wv_t[:, d, cc * VC : (cc + 1) * VC],
                                start=(d == 0),
                                stop=(d == NDT - 1),
                            )
                        nh2 = VC // DH  # heads per chunk (6)
                        o = V[s].rearrange("p (h e) -> p h e", e=EPH)[
                            :, cc * nh2 : (cc + 1) * nh2, 0:DH
                        ]
                        accr = acc.rearrange("p (h e) -> p h e", e=DH)
                        if "bv" in bias_tiles:
                            nc.vector.tensor_add(
                                o,
                                accr,
                                bias_tiles["bv"][
                                    :, cc * VC : (cc + 1) * VC
                                ].rearrange("p (h e) -> p h e", e=DH),
                            )
                        else:
                            nc.scalar.activation(o, accr, AF.Copy)

                    return thunk

                return [mk(0), mk(1)]

            wo_holder = []

            def o_proj_chains(s, zpool_second=False):
                ot_holder = []

                def mk(cc):
                    def thunk():
                        if cc == 0:
                            ot_holder.append(
                                out_pool.tile([128, DM], F32, tag="ostage", name="ostage")
                            )
                        ot = ot_holder[0]
                        pool, tag = (
                            (ps_z, "z") if (zpool_second and cc == 1) else (ps_mm, "proj")
                        )
                        acc = pool.tile(
                            [128, VC], F32, tag=tag, name="oacc",
                            padded_shape=[128, QC],
                        )
                        for d in range(NDT):
                            nc.tensor.matmul(
                                acc,
                                lhsT=ZT[d][:, s * 128 : (s + 1) * 128],
                                rhs=wo_holder[0][:, d, cc * VC : (cc + 1) * VC],
                                start=(d == 0),
                                stop=(d == NDT - 1),
                            )
                        o = ot[:, cc * VC : (cc + 1) * VC]
                        if "bo" in bias_tiles:
                            nc.vector.tensor_add(
                                o, acc, bias_tiles["bo"][:, cc * VC : (cc + 1) * VC]
                            )
                        else:
                            nc.vector.tensor_copy(o, acc)
                        if cc == NVC - 1:
                            nc.sync.dma_start(
                                out=out[s * 128 : (s + 1) * 128, :], in_=ot
                            )

                    return thunk

                return [mk(0), mk(1)]

            # ---- attention unit with PE-bubble fillers ----
            def attn_unit(hp, c, fillers=()):
                fillers = list(fillers)
                zps = {}
                for px in (0, 64):
                    zps[px] = ps_z.tile([128, QC], F32, tag="z", name="z")
                nkb = 4 * (c + 1)  # causal: key blocks 0..nkb-1
                for g in range(0, nkb, 2):  # groups of 2 key-blocks
                    gsz = min(2, nkb - g)
                    # columns [0:doff) of a diagonal block are fully causal-
                    # masked: skip them in scores and PV (ragged-N); stale
                    # contents in skipped columns are never read downstream.
                    doffs = [max(0, (g + j) * 128 - c * QC) for j in range(gsz)]
                    sts = {}
                    for px in (0, 64):
                        sts[px] = ps_st.tile(
                            [128, gsz * QC], F32, tag="st", name="st",
                            padded_shape=[128, 2 * QC],
                        )
                    for j in range(gsz):
                        kb = g + j
                        off = doffs[j]
                        for px in (0, 64):  # head A in partitions 0:64, B in 64:128
                            nc.tensor.matmul(
                                sts[px][:, j * QC + off : (j + 1) * QC],
                                lhsT=KT[hp][px : px + 64, kb * 128 : (kb + 1) * 128],
                                rhs=QT[hp][px : px + 64, c * QC + off : (c + 1) * QC],
                                start=True,
                                stop=True,
                            )
                    # exp exactly the written (causally visible) column ranges;
                    # adjacent full blocks merge into a single instruction
                    eranges = []
                    for j in range(gsz):
                        lo, hi = j * QC + doffs[j], (j + 1) * QC
                        if eranges and eranges[-1][1] == lo:
                            eranges[-1] = (eranges[-1][0], hi)
                        else:
                            eranges.append((lo, hi))
                    pts = {}
                    for px in (0, 64):
                        pt = pt_pool.tile([128, 2 * QC], DT_PV, tag="pt", name="pt")
                        for lo, hi in eranges:
                            nc.scalar.activation(
                                pt[:, lo:hi], sts[px][:, lo:hi], AF.Exp, scale=0.125
                            )
                        pts[px] = pt
                    if fillers:
                        fillers.pop(0)()  # PE filler while ScalarE runs the exp
                    for j in range(gsz):
                        kb = g + j
                        doff = kb * 128 - c * QC
                        off = doffs[j]
                        for px in (0, 64):
                            pt = pts[px]
                            if 0 <= doff < QC:  # diagonal block: 128-wide triangle
                                blk = pt[:, j * QC + doff : j * QC + doff + 128]
                                nc.gpsimd.tensor_mul(blk, blk, wm_t)
                            h = 2 * hp + (1 if px else 0)
                            nc.tensor.matmul(
                                zps[px][0:EPH, off:QC],
                                lhsT=V[kb][:, h * EPH : (h + 1) * EPH],
                                rhs=pt[:, j * QC + off : (j + 1) * QC],
                                start=(kb == 0),
                                stop=(kb == nkb - 1),
                            )
                for f in fillers:
                    f()
                for px in (0, 64):
                    dstage = small.tile([128, QC], F32, tag="dstage", name="dstage")
                    nc.vector.tensor_copy(dstage[0:1, :], zps[px][DH : DH + 1, :])
                    recip = small.tile([128, QC], F32, tag="recip", name="recip")
                    nc.vector.reciprocal_approx_fast(recip[0:1, :], dstage[0:1, :])
                    bcast = small.tile([64, QC], F32, tag="bcast", name="bcast")
                    nc.gpsimd.partition_broadcast(bcast, recip[0:1, :])
                    nc.vector.tensor_mul(
                        ZT[hp][px : px + 64, c * QC : (c + 1) * QC],
                        zps[px][0:64, :],
                        bcast,
                    )

            # ---- phase A/B: transposes + first projections ----
            for s in range(4):
                transpose_s(s)
            for f in qk_proj_chains(0, 0):
                f()
            for f in v_proj_chains(0):
                f()
            transpose_s(4)
            for f in v_proj_chains(1):
                f()
            transpose_s(5)
            for f in v_proj_chains(2):
                f()
            transpose_s(6)
            transpose_s(7)
            for f in v_proj_chains(3):
                f()

            # ---- phase C: attention qc=0 sweep, projections as fillers; the
            # first three qc=1 units ride along to spread the ScalarE exp load
            for hp in range(NHP):
                if hp + 2 < NHP:
                    qk_tiles[hp + 2] = qk_load(hp + 2)
                pend = []
                if hp + 1 < NHP:
                    pend += qk_proj_chains(hp + 1, 0)
                pend += qk_proj_chains(hp, 1)
                if hp < 4:
                    pend += v_proj_chains(4 + hp)
                if hp == 2:  # prefetch O-proj weights mid qc=0 sweep
                    wo_t = w_pool.tile([128, NDT, DM], DT_O, tag="wo", name="wo")
                    nc.scalar.dma_start(
                        out=wo_t, in_=wo.rearrange("(d p) c -> p d c", p=128)
                    )
                    wo_holder.append(wo_t)
                if hp < 3:
                    attn_unit(hp, 0, pend)
                else:
                    attn_unit(hp, 0, pend[:3])
                    attn_unit(hp - 3, 1, pend[3:])

            # ---- attention qc=1 remainder, first-half output proj as fillers ----
            for hp in range(3, NHP):
                pend = []
                if hp < 5:
                    pend += o_proj_chains(2 * (hp - 3))
                    pend += o_proj_chains(2 * (hp - 3) + 1)
                else:
                    pend += o_proj_chains(2 * (hp - 3))
                attn_unit(hp, 1, pend)
            for f in o_proj_chains(3):
                f()

            # ---- phase D: output projection, second half ----
            for s in range(4, NQT):
                for f in o_proj_chains(s, zpool_second=True):
                    f()

    nc.compile()
    return nc


_CACHE = {}


def _get_nc(key, cfg):
    k = (key, cfg)
    if k not in _CACHE:
        _CACHE[k] = build(*key, cfg=cfg)
    return _CACHE[k]


def _prep(inputs, cfg=CFG):
    bf_qk, bf_vproj, bf_pv, bf_o = cfg
    x = np.ascontiguousarray(np.asarray(inputs["normalized_resid_pre"], np.float32))
    dt_qk = _npdt(BF16 if bf_qk else F32R)
    dt_vp = _npdt(BF16 if bf_vproj else F32R)
    dt_pv = _npdt(BF16 if bf_pv else F32R)
    dt_o = _npdt(BF16 if bf_o else F32R)
    dt_mask = _npdt(BF16 if bf_pv else F32)
    wq = np.ascontiguousarray(
        np.asarray(inputs["W_Q"], np.float32).transpose(1, 0, 2).reshape(DM, DM)
    ).astype(dt_qk)
    wk = np.ascontiguousarray(
        np.asarray(inputs["W_K"], np.float32).transpose(1, 0, 2).reshape(DM, DM)
    ).astype(dt_qk)
    wv = np.ascontiguousarray(
        np.asarray(inputs["W_V"], np.float32).transpose(1, 0, 2).reshape(DM, DM)
    ).astype(dt_vp)
    wo = np.ascontiguousarray(
        np.asarray(inputs["W_O"], np.float32).reshape(DM, DM)
    ).astype(dt_o)
    bq = np.asarray(inputs["b_Q"], np.float32).reshape(NDT, 128).T
    bk = np.asarray(inputs["b_K"], np.float32).reshape(NDT, 128).T
    bv = np.asarray(inputs["b_V"], np.float32).reshape(1, DM)
    bo = np.asarray(inputs["b_O"], np.float32).reshape(1, DM)
    jj, uu = np.meshgrid(np.arange(128), np.arange(128), indexing="ij")
    wmask = (uu >= jj).astype(dt_mask)
    key = (
        bool(np.any(bq)),
        bool(np.any(bk)),
        bool(np.any(bv)),
        bool(np.any(bo)),
    )
    common = {
        "wq": wq, "wk": wk, "wv": wv, "wo": wo, "wmask": wmask,
        "identin": np.eye(128, dtype=np.float32),
    }
    if key[0]:
        common["bq"] = np.ascontiguousarray(bq)
    if key[1]:
        common["bk"] = np.ascontiguousarray(bk)
    if key[2]:
        common["bv"] = np.ascontiguousarray(bv)
    if key[3]:
        common["bo"] = np.ascontiguousarray(bo)
    in_maps = [dict(common, x=np.ascontiguousarray(x[b])) for b in range(BATCH)]
    return key, in_maps


def run(inputs, trace=False, cfg=CFG, **kw):
    key, in_maps = _prep(inputs, cfg)
    nc = _get_nc(key, cfg)
    res = run_bass_kernel_spmd(
        nc, in_maps, core_ids=list(range(BATCH)), trace=trace, **kw
    )
    outs = np.stack([res.results[b]["out"] for b in range(BATCH)])
    return outs.astype(np.float32), res


def kernel(**inputs):
    out, _ = run(inputs)
    return out


if __name__ == "__main__":
    rng = np.random.default_rng(0)
    ins = {
        "normalized_resid_pre": rng.standard_normal((8, SEQ, DM)).astype(np.float32),
        "W_Q": (0.02 * rng.standard_normal((NH, DM, DH))).astype(np.float32),
        "b_Q": np.zeros((NH, DH), np.float32),
        "W_K": (0.02 * rng.standard_normal((NH, DM, DH))).astype(np.float32),
        "b_K": np.zeros((NH, DH), np.float32),
        "W_V": (0.02 * rng.standard_normal((NH, DM, DH))).astype(np.float32),
        "b_V": np.zeros((NH, DH), np.float32),
        "W_O": (0.02 * rng.standard_normal((NH, DH, DM))).astype(np.float32),
        "b_O": np.zeros((DM,), np.float32),
    }
    out = kernel(**ins)
    print("kernel output", out.shape, out.dtype, float(np.abs(out).max()))


# revision 21
# speedup vs baseline: 2.1216x; 2.1216x over previous
"""Causal multi-head attention on 8 Trainium2 NeuronCores.

Problem: nn_Attention_46643344835180
  x: [8, 1024, 768], 12 heads x 64 dh, causal softmax attention + output proj.

Sharding: data-parallel over batch (8 batch elements -> 8 cores, no collectives).

Per-core dataflow (batch element b):
  xT = x_b.T                       PE transposes, 6-wide batches      [768, 1024]
  QT = Wq_cat.T @ xT  (+bq)        heads stacked on partitions        [768, 1024]
  KT = Wk_cat.T @ xT  (+bk)                                           [768, 1024]
  V  = x_b @ Wv_cat   (+bv)        + interleaved ones column          [1024, 12*65]
  per head h, query-chunk qc (512):
    S^T[k,q] = KT_h.T @ QT_h          keys on partitions
    P^T = exp(S^T / 8)                ScalarE, batched over 2 key-blocks
    causal: one wide-mask multiply on the partial columns
    z^T[65,512] += [V_h | 1].T @ P^T  row 64 accumulates the denominator
    ZT_h = z^T[0:64] * approx(1/z^T[64])   (reciprocal straight from psum ->
           gpsimd partition_broadcast -> multiply)
  out = ZT.T @ Wo_cat (+bo)                                           [1024, 768]

Scheduling: PE executes in issue order, so projection matmul chains are kept
in a pending queue and issued one chain at a time between attention score
groups (fills the PE bubbles while ScalarE runs exp).  All projection
PSUM->SBUF drains run on GpSimd so ScalarE does exp only.

Dtype config CFG = (bf_qk, bf_vproj, bf_pv, bf_o) picks bf16 vs f32r per stage.
"""

import sys

sys.path.insert(0, "/opt/trn_rl_repo")

import ml_dtypes
import numpy as np

import concourse.bass as bass
import concourse.mybir as mybir
import concourse.tile as tile
from concourse import bacc
from concourse.bass_utils import run_bass_kernel_spmd

F32 = mybir.dt.float32
F32R = mybir.dt.float32r
BF16 = mybir.dt.bfloat16
AF = mybir.ActivationFunctionType

SEQ = 1024
DM = 768
NH = 12
DH = 64
EPH = DH + 1  # 65: head value dim + denominator ones column
BATCH = 8
NQT = SEQ // 128  # 8 seq tiles of 128
NDT = DM // 128  # 6 d_model tiles
QC = 512  # query chunk (moving dim)
NQC = SEQ // QC  # 2
NVC = 2
VC = DM // NVC  # 384
NHP = NH // 2  # 6 head pairs

# (bf_qk, bf_vproj, bf_pv, bf_o)
CFG = (True, True, True, True)


def _npdt(dt):
    return ml_dtypes.bfloat16 if dt == BF16 else np.float32


def build(with_bq, with_bk, with_bv, with_bo, cfg=CFG):
    bf_qk, bf_vproj, bf_pv, bf_o = cfg
    DT_QK = BF16 if bf_qk else F32R  # wq/wk, QT/KT, scores matmul
    DT_VP = BF16 if bf_vproj else F32R  # wv + V-projection compute
    DT_PV = BF16 if bf_pv else F32R  # V storage, P^T, PV matmul
    DT_O = BF16 if bf_o else F32R  # ZT, wo, output matmul
    DT_MASK = BF16 if bf_pv else F32
    # xT feeds both the QK and V projections as a matmul operand, so it must
    # match those weights' dtype
    assert bf_qk == bf_vproj
    DT_X = BF16 if bf_qk else F32R

    nc = bacc.Bacc("TRN2", target_bir_lowering=False, debug=False)

    x = nc.dram_tensor("x", [SEQ, DM], F32R, kind="ExternalInput")
    wq = nc.dram_tensor("wq", [DM, DM], DT_QK, kind="ExternalInput")
    wk = nc.dram_tensor("wk", [DM, DM], DT_QK, kind="ExternalInput")
    wv = nc.dram_tensor("wv", [DM, DM], DT_VP, kind="ExternalInput")
    wo = nc.dram_tensor("wo", [DM, DM], DT_O, kind="ExternalInput")
    wmask = nc.dram_tensor("wmask", [128, 128], DT_MASK, kind="ExternalInput")
    identin = nc.dram_tensor("identin", [128, 128], F32R, kind="ExternalInput")
    bq = bk = bv = bo = None
    if with_bq:
        bq = nc.dram_tensor("bq", [128, NDT], F32, kind="ExternalInput")
    if with_bk:
        bk = nc.dram_tensor("bk", [128, NDT], F32, kind="ExternalInput")
    if with_bv:
        bv = nc.dram_tensor("bv", [1, DM], F32, kind="ExternalInput")
    if with_bo:
        bo = nc.dram_tensor("bo", [1, DM], F32, kind="ExternalInput")
    out = nc.dram_tensor("out", [SEQ, DM], F32, kind="ExternalOutput")

    with tile.TileContext(nc) as tc:
        with (
            tc.tile_pool(name="persist", bufs=1) as persist,
            tc.tile_pool(name="xn", bufs=6) as xn_pool,
            tc.tile_pool(name="wstream", bufs=1) as w_pool,
            tc.tile_pool(name="wqk", bufs=6) as wqk_pool,
            tc.tile_pool(name="pt", bufs=4) as pt_pool,
            tc.tile_pool(name="small", bufs=2) as small,
            tc.tile_pool(name="outst", bufs=2) as out_pool,
            tc.tile_pool(name="ps_st", bufs=2, space="PSUM") as ps_st,
            tc.tile_pool(name="ps_z", bufs=2, space="PSUM") as ps_z,
            tc.tile_pool(name="ps_mm", bufs=2, space="PSUM") as ps_mm,
        ):
            # ---- identity first (warmup gate), then x (longest startup chain).
            # x s-tiles 0-3 on the sync queue, 4-7 on the gpsimd SWDGE queue so
            # the serial per-queue dma_start issue cost (~1us each) halves.
            ident = persist.tile([128, 128], F32R, tag="ident", name="ident")
            nc.sync.dma_start(out=ident, in_=identin[:, :])
            xn = []
            for s in range(NQT):
                t = xn_pool.tile([128, DM], F32R, tag="xn", name="xn")
                eng = nc.sync if s < 4 else nc.gpsimd
                eng.dma_start(out=t, in_=x[s * 128 : (s + 1) * 128, :])
                xn.append(t)

            # HAM warmup: dummy matmuls while the x DMAs land, so the
            # transposes/projections start at 2.4GHz instead of the cold 1.2GHz
            warm_ps = ps_mm.tile(
                [128, 128], F32, tag="proj", name="warm", padded_shape=[128, QC]
            )
            for _ in range(8):
                nc.tensor.matmul(warm_ps, lhsT=ident, rhs=ident, start=True, stop=True)
            def qk_load(hp, eng=None):
                pair = []
                for wsrc in (wq, wk):
                    t = wqk_pool.tile([128, NDT, 128], DT_QK, tag="wqk", name="wqk")
                    (eng or nc.sync).dma_start(
                        out=t,
                        in_=wsrc.rearrange("(d p) c -> p d c", p=128)[
                            :, :, hp * 128 : (hp + 1) * 128
                        ],
                    )
                    pair.append(t)
                return pair

            # first two head-pairs' weights + wv on the scalar queue, ahead of
            # the mask/bias loads (needed within ~10us; sync is busy with x)
            qk_tiles = {0: qk_load(0, nc.scalar), 1: qk_load(1, nc.scalar)}
            wv_t = w_pool.tile([128, NDT, DM], DT_VP, tag="wv", name="wv")
            nc.scalar.dma_start(
                out=wv_t, in_=wv.rearrange("(d p) c -> p d c", p=128)
            )
            wm_t = persist.tile([128, 128], DT_MASK, tag="wmask", name="wmask")
            nc.scalar.dma_start(out=wm_t, in_=wmask[:, :])

            bias_tiles = {}
            if with_bq:
                t = persist.tile([128, NDT], F32, tag="bq", name="bq")
                nc.scalar.dma_start(out=t, in_=bq[:, :])
                bias_tiles["bq"] = t
            if with_bk:
                t = persist.tile([128, NDT], F32, tag="bk", name="bk")
                nc.scalar.dma_start(out=t, in_=bk[:, :])
                bias_tiles["bk"] = t
            if with_bv:
                t = persist.tile([128, DM], F32, tag="bv", name="bv")
                nc.scalar.dma_start(out=t, in_=bv[0:1, :].to_broadcast((128, DM)))
                bias_tiles["bv"] = t
            if with_bo:
                t = persist.tile([128, DM], F32, tag="bo", name="bo")
                nc.scalar.dma_start(out=t, in_=bo[0:1, :].to_broadcast((128, DM)))
                bias_tiles["bo"] = t

            # ---- persistent activations ----
            xT = persist.tile([128, NDT * SEQ], DT_X, tag="xT", name="xT")
            xTv = xT.rearrange("p (d s) -> p d s", d=NDT)
            QT = [
                persist.tile([128, SEQ], DT_QK, tag=f"QT{d}", name=f"QT{d}")
                for d in range(NHP)
            ]
            KT = [
                persist.tile([128, SEQ], DT_QK, tag=f"KT{d}", name=f"KT{d}")
                for d in range(NHP)
            ]
            V = [
                persist.tile([128, NH * EPH], DT_PV, tag=f"V{s}", name=f"V{s}")
                for s in range(NQT)
            ]
            ones_dt = F32 if DT_PV == F32R else DT_PV
            for s in range(NQT):
                # contiguous memset to 1.0; v_proj then overwrites the value
                # columns, leaving 1s only in each head's denominator column
                nc.vector.memset(V[s].bitcast(ones_dt), 1.0)
            ZT = [
                persist.tile([128, SEQ], DT_O, tag=f"ZT{d}", name=f"ZT{d}")
                for d in range(NDT)
            ]

            # ---- transposes: 6 per psum slot, one DVE copy per s-tile ----
            def transpose_s(s):
                pst = ps_st.tile(
                    [128, NDT * 128], F32R, tag="st", name="tp",
                    padded_shape=[128, 2 * QC],
                )
                for d in range(NDT):
                    nc.tensor.transpose(
                        pst[:, d * 128 : (d + 1) * 128],
                        xn[s][:, d * 128 : (d + 1) * 128],
                        ident,
                    )
                nc.vector.tensor_copy(
                    xTv[:, :, s * 128 : (s + 1) * 128],
                    pst.rearrange("p (d q) -> p d q", d=NDT),
                )

            # ---- projection chains (each returns issue-thunks) ----
            def qk_proj_chains(hp, c):
                def mk(widx, dst, bkey):
                    def thunk():
                        acc = ps_mm.tile([128, QC], F32, tag="proj", name="proj")
                        for d in range(NDT):
                            nc.tensor.matmul(
                                acc,
                                lhsT=qk_tiles[hp][widx][:, d, :],
                                rhs=xTv[:, d, c * QC : (c + 1) * QC],
                                start=(d == 0),
                                stop=(d == NDT - 1),
                            )
                        o = dst[hp][:, c * QC : (c + 1) * QC]
                        if bkey in bias_tiles:
                            nc.vector.tensor_scalar_add(
                                o, acc, bias_tiles[bkey][:, hp : hp + 1]
                            )
                        else:
                            nc.vector.tensor_copy(o, acc)

                    return thunk

                return [mk(0, QT, "bq"), mk(1, KT, "bk")]

            def v_proj_chains(s):
                def mk(cc):
                    def thunk():
                        acc = ps_mm.tile(
                            [128, VC], F32, tag="proj", name="vacc",
                            padded_shape=[128, QC],
                        )
                        for d in range(NDT):
                            nc.tensor.matmul(
                                acc,
                                lhsT=xTv[:, d, s * 128 : (s + 1) * 128],
                                rhs=wv_t[:, d, cc * VC : (cc + 1) * VC],
                                start=(d == 0),
                                stop=(d == NDT - 1),
                            )
                        nh2 = VC // DH  # heads per chunk (6)
                        o = V[s].rearrange("p (h e) -> p h e", e=EPH)[
                            :, cc * nh2 : (cc + 1) * nh2, 0:DH
                        ]
                        accr = acc.rearrange("p (h e) -> p h e", e=DH)
                        if "bv" in bias_tiles:
                            nc.vector.tensor_add(
                                o,
                                accr,
                                bias_tiles["bv"][
                                    :, cc * VC : (cc + 1) * VC
                                ].rearrange("p (h e) -> p h e", e=DH),
                            )
                        else:
                            nc.scalar.activation(o, accr, AF.Copy)

                    return thunk

                return [mk(0), mk(1)]

            wo_holder = []

            def o_proj_chains(s, zpool_second=False):
                ot_holder = []

                def mk(cc):
                    def thunk():
                        if cc == 0:
                            ot_holder.append(
                                out_pool.tile([128, DM], F32, tag="ostage", name="ostage")
                            )
                        ot = ot_holder[0]
                        pool, tag = (
                            (ps_z, "z") if (zpool_second and cc == 1) else (ps_mm, "proj")
                        )
                        acc = pool.tile(
                            [128, VC], F32, tag=tag, name="oacc",
                            padded_shape=[128, QC],
                        )
                        for d in range(NDT):
                            nc.tensor.matmul(
                                acc,
                                lhsT=ZT[d][:, s * 128 : (s + 1) * 128],
                                rhs=wo_holder[0][:, d, cc * VC : (cc + 1) * VC],
                                start=(d == 0),
                                stop=(d == NDT - 1),
                            )
                        o = ot[:, cc * VC : (cc + 1) * VC]
                        if "bo" in bias_tiles:
                            nc.vector.tensor_add(
                                o, acc, bias_tiles["bo"][:, cc * VC : (cc + 1) * VC]
                            )
                        else:
                            nc.vector.tensor_copy(o, acc)
                        if cc == NVC - 1:
                            nc.sync.dma_start(
                                out=out[s * 128 : (s + 1) * 128, :], in_=ot
                            )

                    return thunk

                return [mk(0), mk(1)]

            # ---- attention unit with PE-bubble fillers ----
            def attn_unit(hp, c, fillers=()):
                fillers = list(fillers)
                zps = {}
                for px in (0, 64):
                    zps[px] = ps_z.tile([128, QC], F32, tag="z", name="z")
                nkb = 4 * (c + 1)  # causal: key blocks 0..nkb-1
                for g in range(0, nkb, 2):  # groups of 2 key-blocks
                    gsz = min(2, nkb - g)
                    # columns [0:doff) of a diagonal block are fully causal-
                    # masked: skip them in scores and PV (ragged-N); stale
                    # contents in skipped columns are never read downstream.
                    doffs = [max(0, (g + j) * 128 - c * QC) for j in range(gsz)]
                    sts = {}
                    for px in (0, 64):
                        sts[px] = ps_st.tile(
                            [128, gsz * QC], F32, tag="st", name="st",
                            padded_shape=[128, 2 * QC],
                        )
                    for j in range(gsz):
                        kb = g + j
                        off = doffs[j]
                        for px in (0, 64):  # head A in partitions 0:64, B in 64:128
                            nc.tensor.matmul(
                                sts[px][:, j * QC + off : (j + 1) * QC],
                                lhsT=KT[hp][px : px + 64, kb * 128 : (kb + 1) * 128],
                                rhs=QT[hp][px : px + 64, c * QC + off : (c + 1) * QC],
                                start=True,
                                stop=True,
                            )
                    # exp exactly the written (causally visible) column ranges;
                    # adjacent full blocks merge into a single instruction
                    eranges = []
                    for j in range(gsz):
                        lo, hi = j * QC + doffs[j], (j + 1) * QC
                        if eranges and eranges[-1][1] == lo:
                            eranges[-1] = (eranges[-1][0], hi)
                        else:
                            eranges.append((lo, hi))
                    pts = {}
                    for px in (0, 64):
                        pt = pt_pool.tile([128, 2 * QC], DT_PV, tag="pt", name="pt")
                        for lo, hi in eranges:
                            nc.scalar.activation(
                                pt[:, lo:hi], sts[px][:, lo:hi], AF.Exp, scale=0.125
                            )
                        pts[px] = pt
                    if fillers:
                        fillers.pop(0)()  # PE filler while ScalarE runs the exp
                    for j in range(gsz):
                        kb = g + j
                        doff = kb * 128 - c * QC
                        off = doffs[j]
                        for px in (0, 64):
                            pt = pts[px]
                            if 0 <= doff < QC:  # diagonal block: 128-wide triangle
                                blk = pt[:, j * QC + doff : j * QC + doff + 128]
                                nc.vector.tensor_mul(blk, blk, wm_t)
                            h = 2 * hp + (1 if px else 0)
                            nc.tensor.matmul(
                                zps[px][0:EPH, off:QC],
                                lhsT=V[kb][:, h * EPH : (h + 1) * EPH],
                                rhs=pt[:, j * QC + off : (j + 1) * QC],
                                start=(kb == 0),
                                stop=(kb == nkb - 1),
                            )
                for f in fillers:
                    f()
                for px in (0, 64):
                    dstage = small.tile([128, QC], F32, tag="dstage", name="dstage")
                    nc.vector.tensor_copy(dstage[0:1, :], zps[px][DH : DH + 1, :])
                    recip = small.tile([128, QC], F32, tag="recip", name="recip")
                    nc.vector.reciprocal_approx_fast(recip[0:1, :], dstage[0:1, :])
                    bcast = small.tile([64, QC], F32, tag="bcast", name="bcast")
                    nc.gpsimd.partition_broadcast(bcast, recip[0:1, :])
                    nc.vector.tensor_mul(
                        ZT[hp][px : px + 64, c * QC : (c + 1) * QC],
                        zps[px][0:64, :],
                        bcast,
                    )

            # ---- phase A/B: transposes + first projections ----
            for s in range(4):
                transpose_s(s)
            for f in qk_proj_chains(0, 0):
                f()
            for f in v_proj_chains(0):
                f()
            transpose_s(4)
            for f in v_proj_chains(1):
                f()
            transpose_s(5)
            for f in v_proj_chains(2):
                f()
            transpose_s(6)
            transpose_s(7)
            for f in v_proj_chains(3):
                f()

            # ---- phase C: attention qc=0 sweep, projections as fillers; the
            # first three qc=1 units ride along to spread the ScalarE exp load
            for hp in range(NHP):
                if hp + 2 < NHP:
                    qk_tiles[hp + 2] = qk_load(hp + 2)
                pend = []
                if hp + 1 < NHP:
                    pend += qk_proj_chains(hp + 1, 0)
                pend += qk_proj_chains(hp, 1)
                if hp < 4:
                    pend += v_proj_chains(4 + hp)
                if hp == 2:  # prefetch O-proj weights mid qc=0 sweep
                    wo_t = w_pool.tile([128, NDT, DM], DT_O, tag="wo", name="wo")
                    nc.scalar.dma_start(
                        out=wo_t, in_=wo.rearrange("(d p) c -> p d c", p=128)
                    )
                    wo_holder.append(wo_t)
                if hp < 3:
                    attn_unit(hp, 0, pend)
                else:
                    attn_unit(hp, 0, pend[:3])
                    attn_unit(hp - 3, 1, pend[3:])

            # ---- attention qc=1 remainder, first-half output proj as fillers ----
            for hp in range(3, NHP):
                pend = []
                if hp < 5:  # o-proj of qc=0 rows; qc=1 rows must wait phase D
                    pend += o_proj_chains(2 * (hp - 3))
                    pend += o_proj_chains(2 * (hp - 3) + 1)
                attn_unit(hp, 1, pend)

            # ---- phase D: output projection, second half ----
            for s in range(4, NQT):
                for f in o_proj_chains(s, zpool_second=True):
                    f()

    nc.compile()
    return nc


_CACHE = {}


def _get_nc(key, cfg):
    k = (key, cfg)
    if k not in _CACHE:
        _CACHE[k] = build(*key, cfg=cfg)
    return _CACHE[k]


def _prep(inputs, cfg=CFG):
    bf_qk, bf_vproj, bf_pv, bf_o = cfg
    x = np.ascontiguousarray(np.asarray(inputs["normalized_resid_pre"], np.float32))
    dt_qk = _npdt(BF16 if bf_qk else F32R)
    dt_vp = _npdt(BF16 if bf_vproj else F32R)
    dt_pv = _npdt(BF16 if bf_pv else F32R)
    dt_o = _npdt(BF16 if bf_o else F32R)
    dt_mask = _npdt(BF16 if bf_pv else F32)
    wq = np.ascontiguousarray(
        np.asarray(inputs["W_Q"], np.float32).transpose(1, 0, 2).reshape(DM, DM)
    ).astype(dt_qk)
    wk = np.ascontiguousarray(
        np.asarray(inputs["W_K"], np.float32).transpose(1, 0, 2).reshape(DM, DM)
    ).astype(dt_qk)
    wv = np.ascontiguousarray(
        np.asarray(inputs["W_V"], np.float32).transpose(1, 0, 2).reshape(DM, DM)
    ).astype(dt_vp)
    wo = np.ascontiguousarray(
        np.asarray(inputs["W_O"], np.float32).reshape(DM, DM)
    ).astype(dt_o)
    bq = np.asarray(inputs["b_Q"], np.float32).reshape(NDT, 128).T
    bk = np.asarray(inputs["b_K"], np.float32).reshape(NDT, 128).T
    bv = np.asarray(inputs["b_V"], np.float32).reshape(1, DM)
    bo = np.asarray(inputs["b_O"], np.float32).reshape(1, DM)
    jj, uu = np.meshgrid(np.arange(128), np.arange(128), indexing="ij")
    wmask = (uu >= jj).astype(dt_mask)
    key = (
        bool(np.any(bq)),
        bool(np.any(bk)),
        bool(np.any(bv)),
        bool(np.any(bo)),
    )
    common = {
        "wq": wq, "wk": wk, "wv": wv, "wo": wo, "wmask": wmask,
        "identin": np.eye(128, dtype=np.float32),
    }
    if key[0]:
        common["bq"] = np.ascontiguousarray(bq)
    if key[1]:
        common["bk"] = np.ascontiguousarray(bk)
    if key[2]:
        common["bv"] = np.ascontiguousarray(bv)
    if key[3]:
        common["bo"] = np.ascontiguousarray(bo)
    in_maps = [dict(common, x=np.ascontiguousarray(x[b])) for b in range(BATCH)]
    return key, in_maps


def run(inputs, trace=False, cfg=CFG, **kw):
    key, in_maps = _prep(inputs, cfg)
    nc = _get_nc(key, cfg)
    res = run_bass_kernel_spmd(
        nc, in_maps, core_ids=list(range(BATCH)), trace=trace, **kw
    )
    outs = np.stack([res.results[b]["out"] for b in range(BATCH)])
    return outs.astype(np.float32), res


def kernel(**inputs):
    out, _ = run(inputs)
    return out


if __name__ == "__main__":
    rng = np.random.default_rng(0)
    ins = {
        "normalized_resid_pre": rng.standard_normal((8, SEQ, DM)).astype(np.float32),
        "W_Q": (0.02 * rng.standard_normal((NH, DM, DH))).astype(np.float32),
        "b_Q": np.zeros((NH, DH), np.float32),
        "W_K": (0.02 * rng.standard_normal((NH, DM, DH))).astype(np.float32),
        "b_K": np.zeros((NH, DH), np.float32),
        "W_V": (0.02 * rng.standard_normal((NH, DM, DH))).astype(np.float32),
        "b_V": np.zeros((NH, DH), np.float32),
        "W_O": (0.02 * rng.standard_normal((NH, DH, DM))).astype(np.float32),
        "b_O": np.zeros((DM,), np.float32),
    }
    out = kernel(**ins)
    print("kernel output", out.shape, out.dtype, float(np.abs(out).max()))


# revision 27
# speedup vs baseline: 2.1731x; 1.0243x over previous
"""Causal multi-head attention on 8 Trainium2 NeuronCores.

Problem: nn_Attention_46643344835180
  x: [8, 1024, 768], 12 heads x 64 dh, causal softmax attention + output proj.

Sharding: data-parallel over batch (8 batch elements -> 8 cores, no collectives).

Per-core dataflow (batch element b):
  xT = x_b.T                       PE transposes, 6-wide batches      [768, 1024]
  QT = Wq_cat.T @ xT  (+bq)        heads stacked on partitions        [768, 1024]
  KT = Wk_cat.T @ xT  (+bk)                                           [768, 1024]
  V  = x_b @ Wv_cat   (+bv)        + interleaved ones column          [1024, 12*65]
  per head h, query-chunk qc (512):
    S^T[k,q] = KT_h.T @ QT_h          keys on partitions
    P^T = exp(S^T / 8)                ScalarE, batched over 2 key-blocks
    causal: one wide-mask multiply on the partial columns
    z^T[65,512] += [V_h | 1].T @ P^T  row 64 accumulates the denominator
    ZT_h = z^T[0:64] * approx(1/z^T[64])   (reciprocal straight from psum ->
           gpsimd partition_broadcast -> multiply)
  out = ZT.T @ Wo_cat (+bo)                                           [1024, 768]

Scheduling: PE executes in issue order, so projection matmul chains are kept
in a pending queue and issued one chain at a time between attention score
groups (fills the PE bubbles while ScalarE runs exp).  All projection
PSUM->SBUF drains run on GpSimd so ScalarE does exp only.

Dtype config CFG = (bf_qk, bf_vproj, bf_pv, bf_o) picks bf16 vs f32r per stage.
"""

import sys

sys.path.insert(0, "/opt/trn_rl_repo")

import ml_dtypes
import numpy as np

import concourse.bass as bass
import concourse.mybir as mybir
import concourse.tile as tile
from concourse import bacc
from concourse.bass_utils import run_bass_kernel_spmd

F32 = mybir.dt.float32
F32R = mybir.dt.float32r
BF16 = mybir.dt.bfloat16
AF = mybir.ActivationFunctionType

SEQ = 1024
DM = 768
NH = 12
DH = 64
EPH = DH + 1  # 65: head value dim + denominator ones column
BATCH = 8
NQT = SEQ // 128  # 8 seq tiles of 128
NDT = DM // 128  # 6 d_model tiles
QC = 512  # query chunk (moving dim)
NQC = SEQ // QC  # 2
NVC = 2
VC = DM // NVC  # 384
NHP = NH // 2  # 6 head pairs

# (bf_qk, bf_vproj, bf_pv, bf_o)
CFG = (True, True, True, True)


def _npdt(dt):
    return ml_dtypes.bfloat16 if dt == BF16 else np.float32


def build(with_bq, with_bk, with_bv, with_bo, cfg=CFG):
    bf_qk, bf_vproj, bf_pv, bf_o = cfg
    DT_QK = BF16 if bf_qk else F32R  # wq/wk, QT/KT, scores matmul
    DT_VP = BF16 if bf_vproj else F32R  # wv + V-projection compute
    DT_PV = BF16 if bf_pv else F32R  # V storage, P^T, PV matmul
    DT_O = BF16 if bf_o else F32R  # ZT, wo, output matmul
    DT_MASK = BF16 if bf_pv else F32
    # xT feeds both the QK and V projections as a matmul operand, so it must
    # match those weights' dtype
    assert bf_qk == bf_vproj
    DT_X = BF16 if bf_qk else F32R

    nc = bacc.Bacc("TRN2", target_bir_lowering=False, debug=False)

    x = nc.dram_tensor("x", [SEQ, DM], F32, kind="ExternalInput")
    wq = nc.dram_tensor("wq", [DM, DM], DT_QK, kind="ExternalInput")
    wk = nc.dram_tensor("wk", [DM, DM], DT_QK, kind="ExternalInput")
    wv = nc.dram_tensor("wv", [DM, DM], DT_VP, kind="ExternalInput")
    wo = nc.dram_tensor("wo", [DM, DM], DT_O, kind="ExternalInput")
    wmask = nc.dram_tensor("wmask", [128, 128], DT_MASK, kind="ExternalInput")
    identin = nc.dram_tensor("identin", [128, 128], DT_X, kind="ExternalInput")
    bq = bk = bv = bo = None
    if with_bq:
        bq = nc.dram_tensor("bq", [128, NDT], F32, kind="ExternalInput")
    if with_bk:
        bk = nc.dram_tensor("bk", [128, NDT], F32, kind="ExternalInput")
    if with_bv:
        bv = nc.dram_tensor("bv", [1, DM], F32, kind="ExternalInput")
    if with_bo:
        bo = nc.dram_tensor("bo", [1, DM], F32, kind="ExternalInput")
    out = nc.dram_tensor("out", [SEQ, DM], F32, kind="ExternalOutput")

    with tile.TileContext(nc) as tc:
        with (
            tc.tile_pool(name="persist", bufs=1) as persist,
            tc.tile_pool(name="xn", bufs=6) as xn_pool,
            tc.tile_pool(name="wstream", bufs=1) as w_pool,
            tc.tile_pool(name="wqk", bufs=6) as wqk_pool,
            tc.tile_pool(name="pt", bufs=4) as pt_pool,
            tc.tile_pool(name="small", bufs=2) as small,
            tc.tile_pool(name="outst", bufs=2) as out_pool,
            tc.tile_pool(name="ps_st", bufs=2, space="PSUM") as ps_st,
            tc.tile_pool(name="ps_z", bufs=2, space="PSUM") as ps_z,
            tc.tile_pool(name="ps_mm", bufs=2, space="PSUM") as ps_mm,
        ):
            # ---- identity first (warmup gate), then x (longest startup chain).
            # When DT_X is bf16, x is cast f32->bf16 in flight by the gpsimd
            # software-DGE DMA (the only engine that can cast).
            ident = persist.tile([128, 128], DT_X, tag="ident", name="ident")
            nc.sync.dma_start(out=ident, in_=identin[:, :])
            xn = []
            for s in range(NQT):
                t = xn_pool.tile([128, DM], DT_X, tag="xn", name="xn")
                if DT_X == F32R:
                    eng = nc.sync if s < 4 else nc.gpsimd
                else:
                    eng = nc.gpsimd
                eng.dma_start(out=t, in_=x[s * 128 : (s + 1) * 128, :])
                xn.append(t)

            # HAM warmup: dummy matmuls while the x DMAs land, so the
            # transposes/projections start at 2.4GHz instead of the cold 1.2GHz
            warm_ps = ps_mm.tile(
                [128, 128], F32, tag="proj", name="warm", padded_shape=[128, QC]
            )
            for _ in range(20):
                nc.tensor.matmul(warm_ps, lhsT=ident, rhs=ident, start=True, stop=True)
            def qk_load(hp, eng=None):
                pair = []
                for wsrc in (wq, wk):
                    t = wqk_pool.tile([128, NDT, 128], DT_QK, tag="wqk", name="wqk")
                    (eng or nc.sync).dma_start(
                        out=t,
                        in_=wsrc.rearrange("(d p) c -> p d c", p=128)[
                            :, :, hp * 128 : (hp + 1) * 128
                        ],
                    )
                    pair.append(t)
                return pair

            # first two head-pairs' weights + wv on the scalar queue, ahead of
            # the mask/bias loads (needed within ~10us; sync is busy with x)
            qk_tiles = {0: qk_load(0, nc.scalar), 1: qk_load(1, nc.scalar)}
            wv_t = w_pool.tile([128, NDT, DM], DT_VP, tag="wv", name="wv")
            nc.scalar.dma_start(
                out=wv_t, in_=wv.rearrange("(d p) c -> p d c", p=128)
            )
            wm_t = persist.tile([128, 128], DT_MASK, tag="wmask", name="wmask")
            nc.scalar.dma_start(out=wm_t, in_=wmask[:, :])

            bias_tiles = {}
            if with_bq:
                t = persist.tile([128, NDT], F32, tag="bq", name="bq")
                nc.scalar.dma_start(out=t, in_=bq[:, :])
                bias_tiles["bq"] = t
            if with_bk:
                t = persist.tile([128, NDT], F32, tag="bk", name="bk")
                nc.scalar.dma_start(out=t, in_=bk[:, :])
                bias_tiles["bk"] = t
            if with_bv:
                t = persist.tile([128, DM], F32, tag="bv", name="bv")
                nc.scalar.dma_start(out=t, in_=bv[0:1, :].to_broadcast((128, DM)))
                bias_tiles["bv"] = t
            if with_bo:
                t = persist.tile([128, DM], F32, tag="bo", name="bo")
                nc.scalar.dma_start(out=t, in_=bo[0:1, :].to_broadcast((128, DM)))
                bias_tiles["bo"] = t

            # ---- persistent activations ----
            xT = persist.tile([128, NDT * SEQ], DT_X, tag="xT", name="xT")
            xTv = xT.rearrange("p (d s) -> p d s", d=NDT)
            QT = [
                persist.tile([128, SEQ], DT_QK, tag=f"QT{d}", name=f"QT{d}")
                for d in range(NHP)
            ]
            KT = [
                persist.tile([128, SEQ], DT_QK, tag=f"KT{d}", name=f"KT{d}")
                for d in range(NHP)
            ]
            V = [
                persist.tile([128, NH * EPH], DT_PV, tag=f"V{s}", name=f"V{s}")
                for s in range(NQT)
            ]
            ones_dt = F32 if DT_PV == F32R else DT_PV
            for s in range(NQT):
                # contiguous memset to 1.0; v_proj then overwrites the value
                # columns, leaving 1s only in each head's denominator column
                nc.vector.memset(V[s].bitcast(ones_dt), 1.0)
            ZT = [
                persist.tile([128, SEQ], DT_O, tag=f"ZT{d}", name=f"ZT{d}")
                for d in range(NDT)
            ]

            # ---- transposes: 6 per psum slot, one DVE copy per s-tile ----
            def transpose_s(s):
                pst = ps_st.tile(
                    [128, NDT * 128], DT_X, tag="st", name="tp",
                    padded_shape=[128, 2 * QC],
                )
                for d in range(NDT):
                    nc.tensor.transpose(
                        pst[:, d * 128 : (d + 1) * 128],
                        xn[s][:, d * 128 : (d + 1) * 128],
                        ident,
                    )
                nc.vector.tensor_copy(
                    xTv[:, :, s * 128 : (s + 1) * 128],
                    pst.rearrange("p (d q) -> p d q", d=NDT),
                )

            # ---- projection chains (each returns issue-thunks) ----
            def qk_proj_chains(hp, c):
                def mk(widx, dst, bkey):
                    def thunk():
                        acc = ps_mm.tile([128, QC], F32, tag="proj", name="proj")
                        for d in range(NDT):
                            nc.tensor.matmul(
                                acc,
                                lhsT=qk_tiles[hp][widx][:, d, :],
                                rhs=xTv[:, d, c * QC : (c + 1) * QC],
                                start=(d == 0),
                                stop=(d == NDT - 1),
                            )
                        o = dst[hp][:, c * QC : (c + 1) * QC]
                        if bkey in bias_tiles:
                            nc.vector.tensor_scalar_add(
                                o, acc, bias_tiles[bkey][:, hp : hp + 1]
                            )
                        else:
                            nc.vector.tensor_copy(o, acc)

                    return thunk

                return [mk(0, QT, "bq"), mk(1, KT, "bk")]

            def v_proj_chains(s):
                def mk(cc):
                    def thunk():
                        acc = ps_mm.tile(
                            [128, VC], F32, tag="proj", name="vacc",
                            padded_shape=[128, QC],
                        )
                        for d in range(NDT):
                            nc.tensor.matmul(
                                acc,
                                lhsT=xTv[:, d, s * 128 : (s + 1) * 128],
                                rhs=wv_t[:, d, cc * VC : (cc + 1) * VC],
                                start=(d == 0),
                                stop=(d == NDT - 1),
                            )
                        nh2 = VC // DH  # heads per chunk (6)
                        o = V[s].rearrange("p (h e) -> p h e", e=EPH)[
                            :, cc * nh2 : (cc + 1) * nh2, 0:DH
                        ]
                        accr = acc.rearrange("p (h e) -> p h e", e=DH)
                        if "bv" in bias_tiles:
                            nc.vector.tensor_add(
                                o,
                                accr,
                                bias_tiles["bv"][
                                    :, cc * VC : (cc + 1) * VC
                                ].rearrange("p (h e) -> p h e", e=DH),
                            )
                        else:
                            nc.scalar.activation(o, accr, AF.Copy)

                    return thunk

                return [mk(0), mk(1)]

            wo_holder = []

            def o_proj_chains(s, zpool_second=False):
                ot_holder = []

                def mk(cc):
                    def thunk():
                        if cc == 0:
                            ot_holder.append(
                                out_pool.tile([128, DM], F32, tag="ostage", name="ostage")
                            )
                        ot = ot_holder[0]
                        pool, tag = (
                            (ps_z, "z") if (zpool_second and cc == 1) else (ps_mm, "proj")
                        )
                        acc = pool.tile(
                            [128, VC], F32, tag=tag, name="oacc",
                            padded_shape=[128, QC],
                        )
                        for d in range(NDT):
                            nc.tensor.matmul(
                                acc,
                                lhsT=ZT[d][:, s * 128 : (s + 1) * 128],
                                rhs=wo_holder[0][:, d, cc * VC : (cc + 1) * VC],
                                start=(d == 0),
                                stop=(d == NDT - 1),
                            )
                        o = ot[:, cc * VC : (cc + 1) * VC]
                        if "bo" in bias_tiles:
                            nc.vector.tensor_add(
                                o, acc, bias_tiles["bo"][:, cc * VC : (cc + 1) * VC]
                            )
                        else:
                            nc.vector.tensor_copy(o, acc)
                        if cc == NVC - 1:
                            nc.sync.dma_start(
                                out=out[s * 128 : (s + 1) * 128, :], in_=ot
                            )

                    return thunk

                return [mk(0), mk(1)]

            # ---- attention unit with PE-bubble fillers ----
            def attn_unit(hp, c, fillers=()):
                fillers = list(fillers)
                zps = {}
                for px in (0, 64):
                    zps[px] = ps_z.tile([128, QC], F32, tag="z", name="z")
                nkb = 4 * (c + 1)  # causal: key blocks 0..nkb-1
                for g in range(0, nkb, 2):  # groups of 2 key-blocks
                    gsz = min(2, nkb - g)
                    # columns [0:doff) of a diagonal block are fully causal-
                    # masked: skip them in scores and PV (ragged-N); stale
                    # contents in skipped columns are never read downstream.
                    doffs = [max(0, (g + j) * 128 - c * QC) for j in range(gsz)]
                    sts = {}
                    for px in (0, 64):
                        sts[px] = ps_st.tile(
                            [128, gsz * QC], F32, tag="st", name="st",
                            padded_shape=[128, 2 * QC],
                        )
                    for j in range(gsz):
                        kb = g + j
                        off = doffs[j]
                        for px in (0, 64):  # head A in partitions 0:64, B in 64:128
                            nc.tensor.matmul(
                                sts[px][:, j * QC + off : (j + 1) * QC],
                                lhsT=KT[hp][px : px + 64, kb * 128 : (kb + 1) * 128],
                                rhs=QT[hp][px : px + 64, c * QC + off : (c + 1) * QC],
                                start=True,
                                stop=True,
                            )
                    # exp exactly the written (causally visible) column ranges;
                    # adjacent full blocks merge into a single instruction
                    eranges = []
                    for j in range(gsz):
                        lo, hi = j * QC + doffs[j], (j + 1) * QC
                        if eranges and eranges[-1][1] == lo:
                            eranges[-1] = (eranges[-1][0], hi)
                        else:
                            eranges.append((lo, hi))
                    pts = {}
                    for px in (0, 64):
                        pt = pt_pool.tile([128, 2 * QC], DT_PV, tag="pt", name="pt")
                        for lo, hi in eranges:
                            nc.scalar.activation(
                                pt[:, lo:hi], sts[px][:, lo:hi], AF.Exp, scale=0.125
                            )
                        pts[px] = pt
                    if fillers:
                        fillers.pop(0)()  # PE filler while ScalarE runs the exp
                    for j in range(gsz):
                        kb = g + j
                        doff = kb * 128 - c * QC
                        off = doffs[j]
                        for px in (0, 64):
                            pt = pts[px]
                            if 0 <= doff < QC:  # diagonal block: 128-wide triangle
                                blk = pt[:, j * QC + doff : j * QC + doff + 128]
                                nc.vector.tensor_mul(blk, blk, wm_t)
                            h = 2 * hp + (1 if px else 0)
                            nc.tensor.matmul(
                                zps[px][0:EPH, off:QC],
                                lhsT=V[kb][:, h * EPH : (h + 1) * EPH],
                                rhs=pt[:, j * QC + off : (j + 1) * QC],
                                start=(kb == 0),
                                stop=(kb == nkb - 1),
                            )
                for f in fillers:
                    f()
                for px in (0, 64):
                    dstage = small.tile([128, QC], F32, tag="dstage", name="dstage")
                    nc.vector.tensor_copy(dstage[0:1, :], zps[px][DH : DH + 1, :])
                    recip = small.tile([128, QC], F32, tag="recip", name="recip")
                    nc.vector.reciprocal_approx_fast(recip[0:1, :], dstage[0:1, :])
                    bcast = small.tile([64, QC], F32, tag="bcast", name="bcast")
                    nc.gpsimd.partition_broadcast(bcast, recip[0:1, :])
                    nc.vector.tensor_mul(
                        ZT[hp][px : px + 64, c * QC : (c + 1) * QC],
                        zps[px][0:64, :],
                        bcast,
                    )

            # ---- phase A/B: transposes + first projections ----
            for s in range(4):
                transpose_s(s)
            for f in qk_proj_chains(0, 0):
                f()
            for f in v_proj_chains(0):
                f()
            transpose_s(4)
            for f in v_proj_chains(1):
                f()
            transpose_s(5)
            for f in v_proj_chains(2):
                f()
            transpose_s(6)
            transpose_s(7)
            for f in v_proj_chains(3):
                f()

            # ---- phase C: attention qc=0 sweep, projections as fillers; the
            # first three qc=1 units ride along to spread the ScalarE exp load
            for hp in range(NHP):
                if hp + 2 < NHP:
                    qk_tiles[hp + 2] = qk_load(hp + 2)
                pend = []
                if hp + 1 < NHP:
                    pend += qk_proj_chains(hp + 1, 0)
                pend += qk_proj_chains(hp, 1)
                if hp < 4:
                    pend += v_proj_chains(4 + hp)
                if hp == 2:  # prefetch O-proj weights mid qc=0 sweep
                    wo_t = w_pool.tile([128, NDT, DM], DT_O, tag="wo", name="wo")
                    nc.scalar.dma_start(
                        out=wo_t, in_=wo.rearrange("(d p) c -> p d c", p=128)
                    )
                    wo_holder.append(wo_t)
                if hp < 3:
                    attn_unit(hp, 0, pend)
                else:
                    attn_unit(hp, 0, pend[:3])
                    attn_unit(hp - 3, 1, pend[3:])

            # ---- attention qc=1 remainder, first-half output proj as fillers ----
            for hp in range(3, NHP):
                # o-proj of qc=0 rows as fillers; qc=1 rows must wait phase D
                pend = o_proj_chains(0) + o_proj_chains(1) if hp == 3 else (
                    o_proj_chains(hp - 2)
                )
                attn_unit(hp, 1, pend)

            # ---- phase D: output projection, second half ----
            for s in range(4, NQT):
                for f in o_proj_chains(s, zpool_second=True):
                    f()

    nc.compile()
    return nc


_CACHE = {}


def _get_nc(key, cfg):
    k = (key, cfg)
    if k not in _CACHE:
        _CACHE[k] = build(*key, cfg=cfg)
    return _CACHE[k]


def _prep(inputs, cfg=CFG):
    bf_qk, bf_vproj, bf_pv, bf_o = cfg
    x = np.ascontiguousarray(np.asarray(inputs["normalized_resid_pre"], np.float32))
    dt_qk = _npdt(BF16 if bf_qk else F32R)
    dt_vp = _npdt(BF16 if bf_vproj else F32R)
    dt_pv = _npdt(BF16 if bf_pv else F32R)
    dt_o = _npdt(BF16 if bf_o else F32R)
    dt_mask = _npdt(BF16 if bf_pv else F32)
    wq = np.ascontiguousarray(
        np.asarray(inputs["W_Q"], np.float32).transpose(1, 0, 2).reshape(DM, DM)
    ).astype(dt_qk)
    wk = np.ascontiguousarray(
        np.asarray(inputs["W_K"], np.float32).transpose(1, 0, 2).reshape(DM, DM)
    ).astype(dt_qk)
    wv = np.ascontiguousarray(
        np.asarray(inputs["W_V"], np.float32).transpose(1, 0, 2).reshape(DM, DM)
    ).astype(dt_vp)
    wo = np.ascontiguousarray(
        np.asarray(inputs["W_O"], np.float32).reshape(DM, DM)
    ).astype(dt_o)
    bq = np.asarray(inputs["b_Q"], np.float32).reshape(NDT, 128).T
    bk = np.asarray(inputs["b_K"], np.float32).reshape(NDT, 128).T
    bv = np.asarray(inputs["b_V"], np.float32).reshape(1, DM)
    bo = np.asarray(inputs["b_O"], np.float32).reshape(1, DM)
    jj, uu = np.meshgrid(np.arange(128), np.arange(128), indexing="ij")
    wmask = (uu >= jj).astype(dt_mask)
    key = (
        bool(np.any(bq)),
        bool(np.any(bk)),
        bool(np.any(bv)),
        bool(np.any(bo)),
    )
    common = {
        "wq": wq, "wk": wk, "wv": wv, "wo": wo, "wmask": wmask,
        "identin": np.eye(128, dtype=dt_qk),
    }
    if key[0]:
        common["bq"] = np.ascontiguousarray(bq)
    if key[1]:
        common["bk"] = np.ascontiguousarray(bk)
    if key[2]:
        common["bv"] = np.ascontiguousarray(bv)
    if key[3]:
        common["bo"] = np.ascontiguousarray(bo)
    in_maps = [dict(common, x=np.ascontiguousarray(x[b])) for b in range(BATCH)]
    return key, in_maps


def run(inputs, trace=False, cfg=CFG, **kw):
    key, in_maps = _prep(inputs, cfg)
    nc = _get_nc(key, cfg)
    res = run_bass_kernel_spmd(
        nc, in_maps, core_ids=list(range(BATCH)), trace=trace, **kw
    )
    outs = np.stack([res.results[b]["out"] for b in range(BATCH)])
    return outs.astype(np.float32), res


def kernel(**inputs):
    out, _ = run(inputs)
    return out


if __name__ == "__main__":
    rng = np.random.default_rng(0)
    ins = {
        "normalized_resid_pre": rng.standard_normal((8, SEQ, DM)).astype(np.float32),
        "W_Q": (0.02 * rng.standard_normal((NH, DM, DH))).astype(np.float32),
        "b_Q": np.zeros((NH, DH), np.float32),
        "W_K": (0.02 * rng.standard_normal((NH, DM, DH))).astype(np.float32),
        "b_K": np.zeros((NH, DH), np.float32),
        "W_V": (0.02 * rng.standard_normal((NH, DM, DH))).astype(np.float32),
        "b_V": np.zeros((NH, DH), np.float32),
        "W_O": (0.02 * rng.standard_normal((NH, DH, DM))).astype(np.float32),
        "b_O": np.zeros((DM,), np.float32),
    }
    out = kernel(**ins)
    print("kernel output", out.shape, out.dtype, float(np.abs(out).max()))


# revision 29
# speedup vs baseline: 2.1874x; 1.0066x over previous
"""Causal multi-head attention on 8 Trainium2 NeuronCores.

Problem: nn_Attention_46643344835180
  x: [8, 1024, 768], 12 heads x 64 dh, causal softmax attention + output proj.

Sharding: data-parallel over batch (8 batch elements -> 8 cores, no collectives).

Per-core dataflow (batch element b):
  xT = x_b.T                       PE transposes, 6-wide batches      [768, 1024]
  QT = Wq_cat.T @ xT  (+bq)        heads stacked on partitions        [768, 1024]
  KT = Wk_cat.T @ xT  (+bk)                                           [768, 1024]
  V  = x_b @ Wv_cat   (+bv)        + interleaved ones column          [1024, 12*65]
  per head h, query-chunk qc (512):
    S^T[k,q] = KT_h.T @ QT_h          keys on partitions
    P^T = exp(S^T / 8)                ScalarE, batched over 2 key-blocks
    causal: one wide-mask multiply on the partial columns
    z^T[65,512] += [V_h | 1].T @ P^T  row 64 accumulates the denominator
    ZT_h = z^T[0:64] * approx(1/z^T[64])   (reciprocal straight from psum ->
           gpsimd partition_broadcast -> multiply)
  out = ZT.T @ Wo_cat (+bo)                                           [1024, 768]

Scheduling: PE executes in issue order, so projection matmul chains are kept
in a pending queue and issued one chain at a time between attention score
groups (fills the PE bubbles while ScalarE runs exp).  All projection
PSUM->SBUF drains run on GpSimd so ScalarE does exp only.

Dtype config CFG = (bf_qk, bf_vproj, bf_pv, bf_o) picks bf16 vs f32r per stage.
"""

import sys

sys.path.insert(0, "/opt/trn_rl_repo")

import ml_dtypes
import numpy as np

import concourse.bass as bass
import concourse.mybir as mybir
import concourse.tile as tile
from concourse import bacc
from concourse.bass_utils import run_bass_kernel_spmd

F32 = mybir.dt.float32
F32R = mybir.dt.float32r
BF16 = mybir.dt.bfloat16
AF = mybir.ActivationFunctionType

SEQ = 1024
DM = 768
NH = 12
DH = 64
EPH = DH + 1  # 65: head value dim + denominator ones column
BATCH = 8
NQT = SEQ // 128  # 8 seq tiles of 128
NDT = DM // 128  # 6 d_model tiles
QC = 512  # query chunk (moving dim)
NQC = SEQ // QC  # 2
NVC = 2
VC = DM // NVC  # 384
NHP = NH // 2  # 6 head pairs

# (bf_qk, bf_vproj, bf_pv, bf_o)
CFG = (True, True, True, True)


def _npdt(dt):
    return ml_dtypes.bfloat16 if dt == BF16 else np.float32


def build(with_bq, with_bk, with_bv, with_bo, cfg=CFG):
    bf_qk, bf_vproj, bf_pv, bf_o = cfg
    DT_QK = BF16 if bf_qk else F32R  # wq/wk, QT/KT, scores matmul
    DT_VP = BF16 if bf_vproj else F32R  # wv + V-projection compute
    DT_PV = BF16 if bf_pv else F32R  # V storage, P^T, PV matmul
    DT_O = BF16 if bf_o else F32R  # ZT, wo, output matmul
    DT_MASK = BF16 if bf_pv else F32
    # xT feeds both the QK and V projections as a matmul operand, so it must
    # match those weights' dtype
    assert bf_qk == bf_vproj
    DT_X = BF16 if bf_qk else F32R

    nc = bacc.Bacc("TRN2", target_bir_lowering=False, debug=False)

    x = nc.dram_tensor("x", [SEQ, DM], F32, kind="ExternalInput")
    wq = nc.dram_tensor("wq", [DM, DM], DT_QK, kind="ExternalInput")
    wk = nc.dram_tensor("wk", [DM, DM], DT_QK, kind="ExternalInput")
    wv = nc.dram_tensor("wv", [DM, DM], DT_VP, kind="ExternalInput")
    wo = nc.dram_tensor("wo", [DM, DM], DT_O, kind="ExternalInput")
    wmask = nc.dram_tensor("wmask", [128, 128], DT_MASK, kind="ExternalInput")
    identin = nc.dram_tensor("identin", [128, 128], DT_X, kind="ExternalInput")
    bq = bk = bv = bo = None
    if with_bq:
        bq = nc.dram_tensor("bq", [128, NDT], F32, kind="ExternalInput")
    if with_bk:
        bk = nc.dram_tensor("bk", [128, NDT], F32, kind="ExternalInput")
    if with_bv:
        bv = nc.dram_tensor("bv", [1, DM], F32, kind="ExternalInput")
    if with_bo:
        bo = nc.dram_tensor("bo", [1, DM], F32, kind="ExternalInput")
    out = nc.dram_tensor("out", [SEQ, DM], F32, kind="ExternalOutput")

    with tile.TileContext(nc) as tc:
        with (
            tc.tile_pool(name="persist", bufs=1) as persist,
            tc.tile_pool(name="xn", bufs=6) as xn_pool,
            tc.tile_pool(name="wstream", bufs=1) as w_pool,
            tc.tile_pool(name="wqk", bufs=6) as wqk_pool,
            tc.tile_pool(name="pt", bufs=4) as pt_pool,
            tc.tile_pool(name="small", bufs=2) as small,
            tc.tile_pool(name="outst", bufs=2) as out_pool,
            tc.tile_pool(name="ps_st", bufs=2, space="PSUM") as ps_st,
            tc.tile_pool(name="ps_z", bufs=2, space="PSUM") as ps_z,
            tc.tile_pool(name="ps_mm", bufs=2, space="PSUM") as ps_mm,
        ):
            # ---- identity first (warmup gate), then x (longest startup chain).
            # When DT_X is bf16, x is cast f32->bf16 in flight by the gpsimd
            # software-DGE DMA (the only engine that can cast).
            ident = persist.tile([128, 128], DT_X, tag="ident", name="ident")
            nc.sync.dma_start(out=ident, in_=identin[:, :])
            xn = []
            for s in range(NQT):
                t = xn_pool.tile([128, DM], DT_X, tag="xn", name="xn")
                if DT_X == F32R:
                    eng = nc.sync if s < 4 else nc.gpsimd
                else:
                    eng = nc.gpsimd
                eng.dma_start(out=t, in_=x[s * 128 : (s + 1) * 128, :])
                xn.append(t)

            # HAM warmup: dummy matmuls while the x DMAs land, so the
            # transposes/projections start at 2.4GHz instead of the cold 1.2GHz
            warm_ps = ps_mm.tile(
                [128, 128], F32, tag="proj", name="warm", padded_shape=[128, QC]
            )
            for _ in range(20):
                nc.tensor.matmul(warm_ps, lhsT=ident, rhs=ident, start=True, stop=True)
            def qk_load(hp, eng=None):
                pair = []
                for wsrc in (wq, wk):
                    t = wqk_pool.tile([128, NDT, 128], DT_QK, tag="wqk", name="wqk")
                    (eng or nc.sync).dma_start(
                        out=t,
                        in_=wsrc.rearrange("(d p) c -> p d c", p=128)[
                            :, :, hp * 128 : (hp + 1) * 128
                        ],
                    )
                    pair.append(t)
                return pair

            # first two head-pairs' weights + wv on the scalar queue, ahead of
            # the mask/bias loads (needed within ~10us; sync is busy with x)
            qk_tiles = {0: qk_load(0, nc.scalar), 1: qk_load(1, nc.scalar)}
            wv_t = w_pool.tile([128, NDT, DM], DT_VP, tag="wv", name="wv")
            nc.scalar.dma_start(
                out=wv_t, in_=wv.rearrange("(d p) c -> p d c", p=128)
            )
            wm_t = persist.tile([128, 128], DT_MASK, tag="wmask", name="wmask")
            nc.scalar.dma_start(out=wm_t, in_=wmask[:, :])

            bias_tiles = {}
            if with_bq:
                t = persist.tile([128, NDT], F32, tag="bq", name="bq")
                nc.scalar.dma_start(out=t, in_=bq[:, :])
                bias_tiles["bq"] = t
            if with_bk:
                t = persist.tile([128, NDT], F32, tag="bk", name="bk")
                nc.scalar.dma_start(out=t, in_=bk[:, :])
                bias_tiles["bk"] = t
            if with_bv:
                t = persist.tile([128, DM], F32, tag="bv", name="bv")
                nc.scalar.dma_start(out=t, in_=bv[0:1, :].to_broadcast((128, DM)))
                bias_tiles["bv"] = t
            if with_bo:
                t = persist.tile([128, DM], F32, tag="bo", name="bo")
                nc.scalar.dma_start(out=t, in_=bo[0:1, :].to_broadcast((128, DM)))
                bias_tiles["bo"] = t

            # ---- persistent activations ----
            xT = persist.tile([128, NDT * SEQ], DT_X, tag="xT", name="xT")
            xTv = xT.rearrange("p (d s) -> p d s", d=NDT)
            QT = [
                persist.tile([128, SEQ], DT_QK, tag=f"QT{d}", name=f"QT{d}")
                for d in range(NHP)
            ]
            KT = [
                persist.tile([128, SEQ], DT_QK, tag=f"KT{d}", name=f"KT{d}")
                for d in range(NHP)
            ]
            V = [
                persist.tile([128, NH * EPH], DT_PV, tag=f"V{s}", name=f"V{s}")
                for s in range(NQT)
            ]
            ones_dt = F32 if DT_PV == F32R else DT_PV
            for s in range(NQT):
                # contiguous memset to 1.0; v_proj then overwrites the value
                # columns, leaving 1s only in each head's denominator column
                nc.vector.memset(V[s].bitcast(ones_dt), 1.0)
            ZT = [
                persist.tile([128, SEQ], DT_O, tag=f"ZT{d}", name=f"ZT{d}")
                for d in range(NDT)
            ]

            # ---- transposes: 6 per psum slot, one DVE copy per s-tile ----
            def transpose_s(s):
                pst = ps_st.tile(
                    [128, NDT * 128], DT_X, tag="st", name="tp",
                    padded_shape=[128, 2 * QC],
                )
                for d in range(NDT):
                    nc.tensor.transpose(
                        pst[:, d * 128 : (d + 1) * 128],
                        xn[s][:, d * 128 : (d + 1) * 128],
                        ident,
                    )
                nc.vector.tensor_copy(
                    xTv[:, :, s * 128 : (s + 1) * 128],
                    pst.rearrange("p (d q) -> p d q", d=NDT),
                )

            # ---- projection chains (each returns issue-thunks) ----
            def qk_proj_chains(hp, c):
                def mk(widx, dst, bkey):
                    def thunk():
                        acc = ps_mm.tile([128, QC], F32, tag="proj", name="proj")
                        for d in range(NDT):
                            nc.tensor.matmul(
                                acc,
                                lhsT=qk_tiles[hp][widx][:, d, :],
                                rhs=xTv[:, d, c * QC : (c + 1) * QC],
                                start=(d == 0),
                                stop=(d == NDT - 1),
                            )
                        o = dst[hp][:, c * QC : (c + 1) * QC]
                        if bkey in bias_tiles:
                            nc.scalar.activation(
                                o, acc, AF.Identity,
                                bias=bias_tiles[bkey][:, hp : hp + 1],
                            )
                        else:
                            nc.scalar.activation(o, acc, AF.Copy)

                    return thunk

                return [mk(0, QT, "bq"), mk(1, KT, "bk")]

            def v_proj_chains(s):
                def mk(cc):
                    def thunk():
                        acc = ps_mm.tile(
                            [128, VC], F32, tag="proj", name="vacc",
                            padded_shape=[128, QC],
                        )
                        for d in range(NDT):
                            nc.tensor.matmul(
                                acc,
                                lhsT=xTv[:, d, s * 128 : (s + 1) * 128],
                                rhs=wv_t[:, d, cc * VC : (cc + 1) * VC],
                                start=(d == 0),
                                stop=(d == NDT - 1),
                            )
                        nh2 = VC // DH  # heads per chunk (6)
                        o = V[s].rearrange("p (h e) -> p h e", e=EPH)[
                            :, cc * nh2 : (cc + 1) * nh2, 0:DH
                        ]
                        accr = acc.rearrange("p (h e) -> p h e", e=DH)
                        if "bv" in bias_tiles:
                            nc.vector.tensor_add(
                                o,
                                accr,
                                bias_tiles["bv"][
                                    :, cc * VC : (cc + 1) * VC
                                ].rearrange("p (h e) -> p h e", e=DH),
                            )
                        else:
                            nc.scalar.activation(o, accr, AF.Copy)

                    return thunk

                return [mk(0), mk(1)]

            wo_holder = []

            def o_proj_chains(s, zpool_second=False):
                ot_holder = []

                def mk(cc):
                    def thunk():
                        if cc == 0:
                            ot_holder.append(
                                out_pool.tile([128, DM], F32, tag="ostage", name="ostage")
                            )
                        ot = ot_holder[0]
                        pool, tag = (
                            (ps_z, "z") if (zpool_second and cc == 1) else (ps_mm, "proj")
                        )
                        acc = pool.tile(
                            [128, VC], F32, tag=tag, name="oacc",
                            padded_shape=[128, QC],
                        )
                        for d in range(NDT):
                            nc.tensor.matmul(
                                acc,
                                lhsT=ZT[d][:, s * 128 : (s + 1) * 128],
                                rhs=wo_holder[0][:, d, cc * VC : (cc + 1) * VC],
                                start=(d == 0),
                                stop=(d == NDT - 1),
                            )
                        o = ot[:, cc * VC : (cc + 1) * VC]
                        if "bo" in bias_tiles:
                            nc.vector.tensor_add(
                                o, acc, bias_tiles["bo"][:, cc * VC : (cc + 1) * VC]
                            )
                        else:
                            nc.vector.tensor_copy(o, acc)
                        if cc == NVC - 1:
                            nc.sync.dma_start(
                                out=out[s * 128 : (s + 1) * 128, :], in_=ot
                            )

                    return thunk

                return [mk(0), mk(1)]

            # ---- attention unit with PE-bubble fillers ----
            def attn_unit(hp, c, fillers=()):
                fillers = list(fillers)
                zps = {}
                for px in (0, 64):
                    zps[px] = ps_z.tile([128, QC], F32, tag="z", name="z")
                nkb = 4 * (c + 1)  # causal: key blocks 0..nkb-1
                for g in range(0, nkb, 2):  # groups of 2 key-blocks
                    gsz = min(2, nkb - g)
                    # columns [0:doff) of a diagonal block are fully causal-
                    # masked: skip them in scores and PV (ragged-N); stale
                    # contents in skipped columns are never read downstream.
                    doffs = [max(0, (g + j) * 128 - c * QC) for j in range(gsz)]
                    # for the [0, 128] offset pattern, computing block 1's
                    # masked head-columns too merges the exp into a single
                    # instruction (PE rows are cheaper than a ScalarE init);
                    # PV still skips those columns so they are never read
                    soffs = [
                        0 if (doffs == [0, 128] and j == 1) else doffs[j]
                        for j in range(gsz)
                    ]
                    sts = {}
                    for px in (0, 64):
                        sts[px] = ps_st.tile(
                            [128, gsz * QC], F32, tag="st", name="st",
                            padded_shape=[128, 2 * QC],
                        )
                    for j in range(gsz):
                        kb = g + j
                        off = soffs[j]
                        for px in (0, 64):  # head A in partitions 0:64, B in 64:128
                            nc.tensor.matmul(
                                sts[px][:, j * QC + off : (j + 1) * QC],
                                lhsT=KT[hp][px : px + 64, kb * 128 : (kb + 1) * 128],
                                rhs=QT[hp][px : px + 64, c * QC + off : (c + 1) * QC],
                                start=True,
                                stop=True,
                            )
                    # exp exactly the written column ranges; adjacent ranges
                    # merge into a single instruction
                    eranges = []
                    for j in range(gsz):
                        lo, hi = j * QC + soffs[j], (j + 1) * QC
                        if eranges and eranges[-1][1] == lo:
                            eranges[-1] = (eranges[-1][0], hi)
                        else:
                            eranges.append((lo, hi))
                    pts = {}
                    for px in (0, 64):
                        pt = pt_pool.tile([128, 2 * QC], DT_PV, tag="pt", name="pt")
                        for lo, hi in eranges:
                            nc.scalar.activation(
                                pt[:, lo:hi], sts[px][:, lo:hi], AF.Exp, scale=0.125
                            )
                        pts[px] = pt
                    if fillers:
                        fillers.pop(0)()  # PE filler while ScalarE runs the exp
                    for j in range(gsz):
                        kb = g + j
                        doff = kb * 128 - c * QC
                        off = doffs[j]
                        for px in (0, 64):
                            pt = pts[px]
                            if 0 <= doff < QC:  # diagonal block: 128-wide triangle
                                blk = pt[:, j * QC + doff : j * QC + doff + 128]
                                nc.vector.tensor_mul(blk, blk, wm_t)
                            h = 2 * hp + (1 if px else 0)
                            nc.tensor.matmul(
                                zps[px][0:EPH, off:QC],
                                lhsT=V[kb][:, h * EPH : (h + 1) * EPH],
                                rhs=pt[:, j * QC + off : (j + 1) * QC],
                                start=(kb == 0),
                                stop=(kb == nkb - 1),
                            )
                for f in fillers:
                    f()
                for px in (0, 64):
                    dstage = small.tile([128, QC], F32, tag="dstage", name="dstage")
                    nc.vector.tensor_copy(dstage[0:1, :], zps[px][DH : DH + 1, :])
                    recip = small.tile([128, QC], F32, tag="recip", name="recip")
                    nc.vector.reciprocal_approx_fast(recip[0:1, :], dstage[0:1, :])
                    bcast = small.tile([64, QC], F32, tag="bcast", name="bcast")
                    nc.gpsimd.partition_broadcast(bcast, recip[0:1, :])
                    nc.vector.tensor_mul(
                        ZT[hp][px : px + 64, c * QC : (c + 1) * QC],
                        zps[px][0:64, :],
                        bcast,
                    )

            # ---- phase A/B: transposes + first projections ----
            for s in range(4):
                transpose_s(s)
            for f in qk_proj_chains(0, 0):
                f()
            for f in v_proj_chains(0):
                f()
            transpose_s(4)
            for f in v_proj_chains(1):
                f()
            transpose_s(5)
            for f in v_proj_chains(2):
                f()
            transpose_s(6)
            transpose_s(7)
            for f in v_proj_chains(3):
                f()

            # ---- phase C: attention qc=0 sweep, projections as fillers; the
            # first three qc=1 units ride along to spread the ScalarE exp load
            for hp in range(NHP):
                if hp + 2 < NHP:
                    qk_tiles[hp + 2] = qk_load(hp + 2)
                pend = []
                if hp + 1 < NHP:
                    pend += qk_proj_chains(hp + 1, 0)
                pend += qk_proj_chains(hp, 1)
                if hp < 4:
                    pend += v_proj_chains(4 + hp)
                if hp == 2:  # prefetch O-proj weights mid qc=0 sweep
                    wo_t = w_pool.tile([128, NDT, DM], DT_O, tag="wo", name="wo")
                    nc.scalar.dma_start(
                        out=wo_t, in_=wo.rearrange("(d p) c -> p d c", p=128)
                    )
                    wo_holder.append(wo_t)
                if hp < 3:
                    attn_unit(hp, 0, pend)
                else:
                    attn_unit(hp, 0, pend[:3])
                    attn_unit(hp - 3, 1, pend[3:])

            # ---- attention qc=1 remainder, first-half output proj as fillers ----
            for hp in range(3, NHP):
                # o-proj of qc=0 rows as fillers; qc=1 rows must wait phase D
                pend = o_proj_chains(0) + o_proj_chains(1) if hp == 3 else (
                    o_proj_chains(hp - 2)
                )
                attn_unit(hp, 1, pend)

            # ---- phase D: output projection, second half ----
            for s in range(4, NQT):
                for f in o_proj_chains(s, zpool_second=True):
                    f()

    nc.compile()
    return nc


_CACHE = {}


def _get_nc(key, cfg):
    k = (key, cfg)
    if k not in _CACHE:
        _CACHE[k] = build(*key, cfg=cfg)
    return _CACHE[k]


def _prep(inputs, cfg=CFG):
    bf_qk, bf_vproj, bf_pv, bf_o = cfg
    x = np.ascontiguousarray(np.asarray(inputs["normalized_resid_pre"], np.float32))
    dt_qk = _npdt(BF16 if bf_qk else F32R)
    dt_vp = _npdt(BF16 if bf_vproj else F32R)
    dt_pv = _npdt(BF16 if bf_pv else F32R)
    dt_o = _npdt(BF16 if bf_o else F32R)
    dt_mask = _npdt(BF16 if bf_pv else F32)
    wq = np.ascontiguousarray(
        np.asarray(inputs["W_Q"], np.float32).transpose(1, 0, 2).reshape(DM, DM)
    ).astype(dt_qk)
    wk = np.ascontiguousarray(
        np.asarray(inputs["W_K"], np.float32).transpose(1, 0, 2).reshape(DM, DM)
    ).astype(dt_qk)
    wv = np.ascontiguousarray(
        np.asarray(inputs["W_V"], np.float32).transpose(1, 0, 2).reshape(DM, DM)
    ).astype(dt_vp)
    wo = np.ascontiguousarray(
        np.asarray(inputs["W_O"], np.float32).reshape(DM, DM)
    ).astype(dt_o)
    bq = np.asarray(inputs["b_Q"], np.float32).reshape(NDT, 128).T
    bk = np.asarray(inputs["b_K"], np.float32).reshape(NDT, 128).T
    bv = np.asarray(inputs["b_V"], np.float32).reshape(1, DM)
    bo = np.asarray(inputs["b_O"], np.float32).reshape(1, DM)
    jj, uu = np.meshgrid(np.arange(128), np.arange(128), indexing="ij")
    wmask = (uu >= jj).astype(dt_mask)
    key = (
        bool(np.any(bq)),
        bool(np.any(bk)),
        bool(np.any(bv)),
        bool(np.any(bo)),
    )
    common = {
        "wq": wq, "wk": wk, "wv": wv, "wo": wo, "wmask": wmask,
        "identin": np.eye(128, dtype=dt_qk),
    }
    if key[0]:
        common["bq"] = np.ascontiguousarray(bq)
    if key[1]:
        common["bk"] = np.ascontiguousarray(bk)
    if key[2]:
        common["bv"] = np.ascontiguousarray(bv)
    if key[3]:
        common["bo"] = np.ascontiguousarray(bo)
    in_maps = [dict(common, x=np.ascontiguousarray(x[b])) for b in range(BATCH)]
    return key, in_maps


def run(inputs, trace=False, cfg=CFG, **kw):
    key, in_maps = _prep(inputs, cfg)
    nc = _get_nc(key, cfg)
    res = run_bass_kernel_spmd(
        nc, in_maps, core_ids=list(range(BATCH)), trace=trace, **kw
    )
    outs = np.stack([res.results[b]["out"] for b in range(BATCH)])
    return outs.astype(np.float32), res


def kernel(**inputs):
    out, _ = run(inputs)
    return out


if __name__ == "__main__":
    rng = np.random.default_rng(0)
    ins = {
        "normalized_resid_pre": rng.standard_normal((8, SEQ, DM)).astype(np.float32),
        "W_Q": (0.02 * rng.standard_normal((NH, DM, DH))).astype(np.float32),
        "b_Q": np.zeros((NH, DH), np.float32),
        "W_K": (0.02 * rng.standard_normal((NH, DM, DH))).astype(np.float32),
        "b_K": np.zeros((NH, DH), np.float32),
        "W_V": (0.02 * rng.standard_normal((NH, DM, DH))).astype(np.float32),
        "b_V": np.zeros((NH, DH), np.float32),
        "W_O": (0.02 * rng.standard_normal((NH, DH, DM))).astype(np.float32),
        "b_O": np.zeros((DM,), np.float32),
    }
    out = kernel(**ins)
    print("kernel output", out.shape, out.dtype, float(np.abs(out).max()))


# revision 32
# speedup vs baseline: 2.2466x; 1.0271x over previous
"""Causal multi-head attention on 8 Trainium2 NeuronCores.

Problem: nn_Attention_46643344835180
  x: [8, 1024, 768], 12 heads x 64 dh, causal softmax attention + output proj.

Sharding: data-parallel over batch (8 batch elements -> 8 cores, no collectives).

Per-core dataflow (batch element b):
  xT = x_b.T                       PE transposes, 6-wide batches      [768, 1024]
  QT = Wq_cat.T @ xT  (+bq)        heads stacked on partitions        [768, 1024]
  KT = Wk_cat.T @ xT  (+bk)                                           [768, 1024]
  V  = x_b @ Wv_cat   (+bv)        + interleaved ones column          [1024, 12*65]
  per head h, query-chunk qc (512):
    S^T[k,q] = KT_h.T @ QT_h          keys on partitions
    P^T = exp(S^T / 8)                ScalarE, batched over 2 key-blocks
    causal: one wide-mask multiply on the partial columns
    z^T[65,512] += [V_h | 1].T @ P^T  row 64 accumulates the denominator
    ZT_h = z^T[0:64] * approx(1/z^T[64])   (reciprocal straight from psum ->
           gpsimd partition_broadcast -> multiply)
  out = ZT.T @ Wo_cat (+bo)                                           [1024, 768]

Scheduling: PE executes in issue order, so projection matmul chains are kept
in a pending queue and issued one chain at a time between attention score
groups (fills the PE bubbles while ScalarE runs exp).  All projection
PSUM->SBUF drains run on GpSimd so ScalarE does exp only.

Dtype config CFG = (bf_qk, bf_vproj, bf_pv, bf_o) picks bf16 vs f32r per stage.
"""

import sys

sys.path.insert(0, "/opt/trn_rl_repo")

import ml_dtypes
import numpy as np

import concourse.bass as bass
import concourse.mybir as mybir
import concourse.tile as tile
from concourse import bacc
from concourse.bass_utils import run_bass_kernel_spmd

F32 = mybir.dt.float32
F32R = mybir.dt.float32r
BF16 = mybir.dt.bfloat16
AF = mybir.ActivationFunctionType

SEQ = 1024
DM = 768
NH = 12
DH = 64
EPH = DH + 1  # 65: head value dim + denominator ones column
BATCH = 8
NQT = SEQ // 128  # 8 seq tiles of 128
NDT = DM // 128  # 6 d_model tiles
QC = 512  # query chunk (moving dim)
NQC = SEQ // QC  # 2
NVC = 2
VC = DM // NVC  # 384
NHP = NH // 2  # 6 head pairs

# (bf_qk, bf_vproj, bf_pv, bf_o)
CFG = (True, True, True, True)


def _npdt(dt):
    return ml_dtypes.bfloat16 if dt == BF16 else np.float32


def build(with_bq, with_bk, with_bv, with_bo, cfg=CFG):
    bf_qk, bf_vproj, bf_pv, bf_o = cfg
    DT_QK = BF16 if bf_qk else F32R  # wq/wk, QT/KT, scores matmul
    DT_VP = BF16 if bf_vproj else F32R  # wv + V-projection compute
    DT_PV = BF16 if bf_pv else F32R  # V storage, P^T, PV matmul
    DT_O = BF16 if bf_o else F32R  # ZT, wo, output matmul
    DT_MASK = BF16 if bf_pv else F32
    # xT feeds both the QK and V projections as a matmul operand, so it must
    # match those weights' dtype
    assert bf_qk == bf_vproj
    DT_X = BF16 if bf_qk else F32R

    nc = bacc.Bacc("TRN2", target_bir_lowering=False, debug=False)

    x = nc.dram_tensor("x", [SEQ, DM], F32, kind="ExternalInput")
    wq = nc.dram_tensor("wq", [DM, DM], DT_QK, kind="ExternalInput")
    wk = nc.dram_tensor("wk", [DM, DM], DT_QK, kind="ExternalInput")
    wv = nc.dram_tensor("wv", [DM, DM], DT_VP, kind="ExternalInput")
    wo = nc.dram_tensor("wo", [DM, DM], DT_O, kind="ExternalInput")
    wmask = nc.dram_tensor("wmask", [128, 128], DT_MASK, kind="ExternalInput")
    identin = nc.dram_tensor("identin", [128, 128], DT_X, kind="ExternalInput")
    bq = bk = bv = bo = None
    if with_bq:
        bq = nc.dram_tensor("bq", [128, NDT], F32, kind="ExternalInput")
    if with_bk:
        bk = nc.dram_tensor("bk", [128, NDT], F32, kind="ExternalInput")
    if with_bv:
        bv = nc.dram_tensor("bv", [1, DM], F32, kind="ExternalInput")
    if with_bo:
        bo = nc.dram_tensor("bo", [1, DM], F32, kind="ExternalInput")
    out = nc.dram_tensor("out", [SEQ, DM], F32, kind="ExternalOutput")

    with tile.TileContext(nc) as tc:
        with (
            tc.tile_pool(name="persist", bufs=1) as persist,
            tc.tile_pool(name="xn", bufs=6) as xn_pool,
            tc.tile_pool(name="wstream", bufs=1) as w_pool,
            tc.tile_pool(name="wqk", bufs=6) as wqk_pool,
            tc.tile_pool(name="pt", bufs=4) as pt_pool,
            tc.tile_pool(name="small", bufs=2) as small,
            tc.tile_pool(name="outst", bufs=2) as out_pool,
            tc.tile_pool(name="ps_st", bufs=2, space="PSUM") as ps_st,
            tc.tile_pool(name="ps_z", bufs=2, space="PSUM") as ps_z,
            tc.tile_pool(name="ps_mm", bufs=2, space="PSUM") as ps_mm,
        ):
            # ---- identity first (warmup gate), then x (longest startup chain).
            # When DT_X is bf16, x is cast f32->bf16 in flight by the gpsimd
            # software-DGE DMA (the only engine that can cast).
            ident = persist.tile([128, 128], DT_X, tag="ident", name="ident")
            nc.sync.dma_start(out=ident, in_=identin[:, :])
            xn = []
            for s in range(NQT):
                t = xn_pool.tile([128, DM], DT_X, tag="xn", name="xn")
                if DT_X == F32R:
                    eng = nc.sync if s < 4 else nc.gpsimd
                else:
                    eng = nc.gpsimd
                eng.dma_start(out=t, in_=x[s * 128 : (s + 1) * 128, :])
                xn.append(t)

            # HAM warmup: dummy matmuls while the x DMAs land, so the
            # transposes/projections start at 2.4GHz instead of the cold 1.2GHz
            warm_ps = ps_mm.tile(
                [128, 128], F32, tag="proj", name="warm", padded_shape=[128, QC]
            )
            for _ in range(20):
                nc.tensor.matmul(warm_ps, lhsT=ident, rhs=ident, start=True, stop=True)
            def qk_load(hp, eng=None):
                pair = []
                for wsrc in (wq, wk):
                    t = wqk_pool.tile([128, NDT, 128], DT_QK, tag="wqk", name="wqk")
                    (eng or nc.sync).dma_start(
                        out=t,
                        in_=wsrc.rearrange("(d p) c -> p d c", p=128)[
                            :, :, hp * 128 : (hp + 1) * 128
                        ],
                    )
                    pair.append(t)
                return pair

            # first two head-pairs' weights + wv on the scalar queue, ahead of
            # the mask/bias loads (needed within ~10us; sync is busy with x)
            qk_tiles = {0: qk_load(0, nc.scalar), 1: qk_load(1, nc.scalar)}
            wv_t = w_pool.tile([128, NDT, DM], DT_VP, tag="wv", name="wv")
            nc.scalar.dma_start(
                out=wv_t, in_=wv.rearrange("(d p) c -> p d c", p=128)
            )
            wm_t = persist.tile([128, 128], DT_MASK, tag="wmask", name="wmask")
            nc.scalar.dma_start(out=wm_t, in_=wmask[:, :])

            bias_tiles = {}
            if with_bq:
                t = persist.tile([128, NDT], F32, tag="bq", name="bq")
                nc.scalar.dma_start(out=t, in_=bq[:, :])
                bias_tiles["bq"] = t
            if with_bk:
                t = persist.tile([128, NDT], F32, tag="bk", name="bk")
                nc.scalar.dma_start(out=t, in_=bk[:, :])
                bias_tiles["bk"] = t
            if with_bv:
                t = persist.tile([128, DM], F32, tag="bv", name="bv")
                nc.scalar.dma_start(out=t, in_=bv[0:1, :].to_broadcast((128, DM)))
                bias_tiles["bv"] = t
            if with_bo:
                t = persist.tile([128, DM], F32, tag="bo", name="bo")
                nc.scalar.dma_start(out=t, in_=bo[0:1, :].to_broadcast((128, DM)))
                bias_tiles["bo"] = t

            # ---- persistent activations ----
            xT = persist.tile([128, NDT * SEQ], DT_X, tag="xT", name="xT")
            xTv = xT.rearrange("p (d s) -> p d s", d=NDT)
            QT = [
                persist.tile([128, SEQ], DT_QK, tag=f"QT{d}", name=f"QT{d}")
                for d in range(NHP)
            ]
            KT = [
                persist.tile([128, SEQ], DT_QK, tag=f"KT{d}", name=f"KT{d}")
                for d in range(NHP)
            ]
            V = [
                persist.tile([128, NH * EPH], DT_PV, tag=f"V{s}", name=f"V{s}")
                for s in range(NQT)
            ]
            ones_dt = F32 if DT_PV == F32R else DT_PV
            for s in range(NQT):
                # contiguous memset to 1.0; v_proj then overwrites the value
                # columns, leaving 1s only in each head's denominator column
                nc.vector.memset(V[s].bitcast(ones_dt), 1.0)
            ZT = [
                persist.tile([128, SEQ], DT_O, tag=f"ZT{d}", name=f"ZT{d}")
                for d in range(NDT)
            ]

            # ---- transposes: 6 per psum slot, one DVE copy per s-tile ----
            def transpose_s(s):
                pst = ps_st.tile(
                    [128, NDT * 128], DT_X, tag="st", name="tp",
                    padded_shape=[128, 2 * QC],
                )
                for d in range(NDT):
                    nc.tensor.transpose(
                        pst[:, d * 128 : (d + 1) * 128],
                        xn[s][:, d * 128 : (d + 1) * 128],
                        ident,
                    )
                nc.vector.tensor_copy(
                    xTv[:, :, s * 128 : (s + 1) * 128],
                    pst.rearrange("p (d q) -> p d q", d=NDT),
                )

            # ---- projection chains (each returns issue-thunks) ----
            def qk_proj_chains(hp, c):
                def mk(widx, dst, bkey):
                    def thunk():
                        acc = ps_mm.tile([128, QC], F32, tag="proj", name="proj")
                        for d in range(NDT):
                            nc.tensor.matmul(
                                acc,
                                lhsT=qk_tiles[hp][widx][:, d, :],
                                rhs=xTv[:, d, c * QC : (c + 1) * QC],
                                start=(d == 0),
                                stop=(d == NDT - 1),
                            )
                        o = dst[hp][:, c * QC : (c + 1) * QC]
                        if bkey in bias_tiles:
                            nc.scalar.activation(
                                o, acc, AF.Identity,
                                bias=bias_tiles[bkey][:, hp : hp + 1],
                            )
                        else:
                            nc.scalar.activation(o, acc, AF.Copy)

                    return thunk

                return [mk(0, QT, "bq"), mk(1, KT, "bk")]

            def v_proj_chains(s):
                def mk(cc):
                    def thunk():
                        acc = ps_mm.tile(
                            [128, VC], F32, tag="proj", name="vacc",
                            padded_shape=[128, QC],
                        )
                        for d in range(NDT):
                            nc.tensor.matmul(
                                acc,
                                lhsT=xTv[:, d, s * 128 : (s + 1) * 128],
                                rhs=wv_t[:, d, cc * VC : (cc + 1) * VC],
                                start=(d == 0),
                                stop=(d == NDT - 1),
                            )
                        nh2 = VC // DH  # heads per chunk (6)
                        o = V[s].rearrange("p (h e) -> p h e", e=EPH)[
                            :, cc * nh2 : (cc + 1) * nh2, 0:DH
                        ]
                        accr = acc.rearrange("p (h e) -> p h e", e=DH)
                        if "bv" in bias_tiles:
                            nc.vector.tensor_add(
                                o,
                                accr,
                                bias_tiles["bv"][
                                    :, cc * VC : (cc + 1) * VC
                                ].rearrange("p (h e) -> p h e", e=DH),
                            )
                        else:
                            nc.scalar.activation(o, accr, AF.Copy)

                    return thunk

                return [mk(0), mk(1)]

            wo_holder = []
            # phase-D accumulators rotate through all three psum pools so a
            # drain in one ring never stalls the next chain
            _opool_rr = [(ps_mm, "proj"), (ps_st, "st"), (ps_z, "z")]
            _opool_idx = [0]

            def o_proj_chains(s, rotate_pools=False):
                ot_holder = []

                def mk(cc):
                    def thunk():
                        if cc == 0:
                            ot_holder.append(
                                out_pool.tile([128, DM], F32, tag="ostage", name="ostage")
                            )
                        ot = ot_holder[0]
                        if rotate_pools:
                            pool, tag = _opool_rr[_opool_idx[0] % 3]
                            _opool_idx[0] += 1
                        else:
                            pool, tag = ps_mm, "proj"
                        kw = (
                            {"padded_shape": [128, 2 * QC]}
                            if tag == "st"
                            else {"padded_shape": [128, QC]}
                        )
                        acc = pool.tile([128, VC], F32, tag=tag, name="oacc", **kw)
                        for d in range(NDT):
                            nc.tensor.matmul(
                                acc,
                                lhsT=ZT[d][:, s * 128 : (s + 1) * 128],
                                rhs=wo_holder[0][:, d, cc * VC : (cc + 1) * VC],
                                start=(d == 0),
                                stop=(d == NDT - 1),
                            )
                        o = ot[:, cc * VC : (cc + 1) * VC]
                        if "bo" in bias_tiles:
                            nc.vector.tensor_add(
                                o, acc, bias_tiles["bo"][:, cc * VC : (cc + 1) * VC]
                            )
                        else:
                            nc.vector.tensor_copy(o, acc)
                        # per-chunk output DMA: shorter drain after the last copy
                        nc.sync.dma_start(
                            out=out[s * 128 : (s + 1) * 128, cc * VC : (cc + 1) * VC],
                            in_=ot[:, cc * VC : (cc + 1) * VC],
                        )

                    return thunk

                return [mk(0), mk(1)]

            # ---- attention unit with PE-bubble fillers ----
            def attn_unit(hp, c, fillers=()):
                fillers = list(fillers)
                zps = {}
                for px in (0, 64):
                    zps[px] = ps_z.tile([128, QC], F32, tag="z", name="z")
                nkb = 4 * (c + 1)  # causal: key blocks 0..nkb-1
                for g in range(0, nkb, 2):  # groups of 2 key-blocks
                    gsz = min(2, nkb - g)
                    # columns [0:doff) of a diagonal block are fully causal-
                    # masked: skip them in scores and PV (ragged-N); stale
                    # contents in skipped columns are never read downstream.
                    doffs = [max(0, (g + j) * 128 - c * QC) for j in range(gsz)]
                    # for the [0, 128] offset pattern, computing block 1's
                    # masked head-columns too merges the exp into a single
                    # instruction (PE rows are cheaper than a ScalarE init);
                    # PV still skips those columns so they are never read
                    soffs = [
                        0 if (doffs == [0, 128] and j == 1) else doffs[j]
                        for j in range(gsz)
                    ]
                    sts = {}
                    for px in (0, 64):
                        sts[px] = ps_st.tile(
                            [128, gsz * QC], F32, tag="st", name="st",
                            padded_shape=[128, 2 * QC],
                        )
                    for j in range(gsz):
                        kb = g + j
                        off = soffs[j]
                        for px in (0, 64):  # head A in partitions 0:64, B in 64:128
                            nc.tensor.matmul(
                                sts[px][:, j * QC + off : (j + 1) * QC],
                                lhsT=KT[hp][px : px + 64, kb * 128 : (kb + 1) * 128],
                                rhs=QT[hp][px : px + 64, c * QC + off : (c + 1) * QC],
                                start=True,
                                stop=True,
                            )
                    # exp exactly the written column ranges; adjacent ranges
                    # merge into a single instruction
                    eranges = []
                    for j in range(gsz):
                        lo, hi = j * QC + soffs[j], (j + 1) * QC
                        if eranges and eranges[-1][1] == lo:
                            eranges[-1] = (eranges[-1][0], hi)
                        else:
                            eranges.append((lo, hi))
                    pts = {}
                    for px in (0, 64):
                        pt = pt_pool.tile([128, 2 * QC], DT_PV, tag="pt", name="pt")
                        for lo, hi in eranges:
                            nc.scalar.activation(
                                pt[:, lo:hi], sts[px][:, lo:hi], AF.Exp, scale=0.125
                            )
                        pts[px] = pt
                    if fillers:
                        fillers.pop(0)()  # PE filler while ScalarE runs the exp
                    for j in range(gsz):
                        kb = g + j
                        doff = kb * 128 - c * QC
                        off = doffs[j]
                        for px in (0, 64):
                            pt = pts[px]
                            if 0 <= doff < QC:  # diagonal block: 128-wide triangle
                                blk = pt[:, j * QC + doff : j * QC + doff + 128]
                                nc.vector.tensor_mul(blk, blk, wm_t)
                            h = 2 * hp + (1 if px else 0)
                            nc.tensor.matmul(
                                zps[px][0:EPH, off:QC],
                                lhsT=V[kb][:, h * EPH : (h + 1) * EPH],
                                rhs=pt[:, j * QC + off : (j + 1) * QC],
                                start=(kb == 0),
                                stop=(kb == nkb - 1),
                            )
                for f in fillers:
                    f()
                for px in (0, 64):
                    dstage = small.tile([128, QC], F32, tag="dstage", name="dstage")
                    nc.vector.tensor_copy(dstage[0:1, :], zps[px][DH : DH + 1, :])
                    recip = small.tile([128, QC], F32, tag="recip", name="recip")
                    nc.vector.reciprocal_approx_fast(recip[0:1, :], dstage[0:1, :])
                    bcast = small.tile([64, QC], F32, tag="bcast", name="bcast")
                    nc.gpsimd.partition_broadcast(bcast, recip[0:1, :])
                    nc.vector.tensor_mul(
                        ZT[hp][px : px + 64, c * QC : (c + 1) * QC],
                        zps[px][0:64, :],
                        bcast,
                    )

            # ---- phase A/B: transposes + first projections ----
            for s in range(4):
                transpose_s(s)
            for f in qk_proj_chains(0, 0):
                f()
            for f in v_proj_chains(0):
                f()
            transpose_s(4)
            for f in v_proj_chains(1):
                f()
            transpose_s(5)
            for f in v_proj_chains(2):
                f()
            transpose_s(6)
            transpose_s(7)
            for f in v_proj_chains(3):
                f()

            # ---- phase C: attention qc=0 sweep, projections as fillers; the
            # first three qc=1 units ride along to spread the ScalarE exp load
            for hp in range(NHP):
                if hp + 2 < NHP:
                    qk_tiles[hp + 2] = qk_load(hp + 2)
                pend = []
                if hp + 1 < NHP:
                    pend += qk_proj_chains(hp + 1, 0)
                pend += qk_proj_chains(hp, 1)
                if hp < 4:
                    pend += v_proj_chains(4 + hp)
                if hp == 2:  # prefetch O-proj weights mid qc=0 sweep
                    wo_t = w_pool.tile([128, NDT, DM], DT_O, tag="wo", name="wo")
                    nc.scalar.dma_start(
                        out=wo_t, in_=wo.rearrange("(d p) c -> p d c", p=128)
                    )
                    wo_holder.append(wo_t)
                if hp < 3:
                    attn_unit(hp, 0, pend)
                else:
                    attn_unit(hp, 0, pend[:3])
                    attn_unit(hp - 3, 1, pend[3:])

            # ---- attention qc=1 remainder, first-half output proj as fillers ----
            # o-proj of qc=0 rows as fillers (3/3/2 split); qc=1 rows wait phase D
            ochains = []
            for s in range(4):
                ochains += o_proj_chains(s)
            attn_unit(3, 1, ochains[0:3])
            attn_unit(4, 1, ochains[3:6])
            attn_unit(5, 1, ochains[6:8])

            # ---- phase D: output projection, second half ----
            for s in range(4, NQT):
                for f in o_proj_chains(s, rotate_pools=True):
                    f()

    nc.compile()
    return nc


_CACHE = {}


def _get_nc(key, cfg):
    k = (key, cfg)
    if k not in _CACHE:
        _CACHE[k] = build(*key, cfg=cfg)
    return _CACHE[k]


def _prep(inputs, cfg=CFG):
    bf_qk, bf_vproj, bf_pv, bf_o = cfg
    x = np.ascontiguousarray(np.asarray(inputs["normalized_resid_pre"], np.float32))
    dt_qk = _npdt(BF16 if bf_qk else F32R)
    dt_vp = _npdt(BF16 if bf_vproj else F32R)
    dt_pv = _npdt(BF16 if bf_pv else F32R)
    dt_o = _npdt(BF16 if bf_o else F32R)
    dt_mask = _npdt(BF16 if bf_pv else F32)
    wq = np.ascontiguousarray(
        np.asarray(inputs["W_Q"], np.float32).transpose(1, 0, 2).reshape(DM, DM)
    ).astype(dt_qk)
    wk = np.ascontiguousarray(
        np.asarray(inputs["W_K"], np.float32).transpose(1, 0, 2).reshape(DM, DM)
    ).astype(dt_qk)
    wv = np.ascontiguousarray(
        np.asarray(inputs["W_V"], np.float32).transpose(1, 0, 2).reshape(DM, DM)
    ).astype(dt_vp)
    wo = np.ascontiguousarray(
        np.asarray(inputs["W_O"], np.float32).reshape(DM, DM)
    ).astype(dt_o)
    bq = np.asarray(inputs["b_Q"], np.float32).reshape(NDT, 128).T
    bk = np.asarray(inputs["b_K"], np.float32).reshape(NDT, 128).T
    bv = np.asarray(inputs["b_V"], np.float32).reshape(1, DM)
    bo = np.asarray(inputs["b_O"], np.float32).reshape(1, DM)
    jj, uu = np.meshgrid(np.arange(128), np.arange(128), indexing="ij")
    wmask = (uu >= jj).astype(dt_mask)
    key = (
        bool(np.any(bq)),
        bool(np.any(bk)),
        bool(np.any(bv)),
        bool(np.any(bo)),
    )
    common = {
        "wq": wq, "wk": wk, "wv": wv, "wo": wo, "wmask": wmask,
        "identin": np.eye(128, dtype=dt_qk),
    }
    if key[0]:
        common["bq"] = np.ascontiguousarray(bq)
    if key[1]:
        common["bk"] = np.ascontiguousarray(bk)
    if key[2]:
        common["bv"] = np.ascontiguousarray(bv)
    if key[3]:
        common["bo"] = np.ascontiguousarray(bo)
    in_maps = [dict(common, x=np.ascontiguousarray(x[b])) for b in range(BATCH)]
    return key, in_maps


def run(inputs, trace=False, cfg=CFG, **kw):
    key, in_maps = _prep(inputs, cfg)
    nc = _get_nc(key, cfg)
    res = run_bass_kernel_spmd(
        nc, in_maps, core_ids=list(range(BATCH)), trace=trace, **kw
    )
    outs = np.stack([res.results[b]["out"] for b in range(BATCH)])
    return outs.astype(np.float32), res


def kernel(**inputs):
    out, _ = run(inputs)
    return out


if __name__ == "__main__":
    rng = np.random.default_rng(0)
    ins = {
        "normalized_resid_pre": rng.standard_normal((8, SEQ, DM)).astype(np.float32),
        "W_Q": (0.02 * rng.standard_normal((NH, DM, DH))).astype(np.float32),
        "b_Q": np.zeros((NH, DH), np.float32),
        "W_K": (0.02 * rng.standard_normal((NH, DM, DH))).astype(np.float32),
        "b_K": np.zeros((NH, DH), np.float32),
        "W_V": (0.02 * rng.standard_normal((NH, DM, DH))).astype(np.float32),
        "b_V": np.zeros((NH, DH), np.float32),
        "W_O": (0.02 * rng.standard_normal((NH, DH, DM))).astype(np.float32),
        "b_O": np.zeros((DM,), np.float32),
    }
    out = kernel(**ins)
    print("kernel output", out.shape, out.dtype, float(np.abs(out).max()))
